# revision 27
# baseline (speedup 1.0000x reference)
"""Trainium2 Bass kernel for EvalNet (EmbeddingBag + MLP + bucketed heads).

Strategy (pure data parallel over 8 cores, batch dim sharded):
  The EmbeddingBag-sum  h[b] = sum_j emb_w[x[b,j]]  is reformulated as a
  dense matmul  h^T = W_pad^T @ C^T  where C[b, v] counts how many of the
  32 slots of sample b hold index v.  Because emb_w's padding row (768)
  is zero and rows 769+ don't exist, the effective vocab is 768 = 6*128
  when bias1 == 0 (the usual case); with a nonzero bias1 a 7th chunk
  carries bias1 in row 895 and a constant-1 column is injected.

  Per 128-sample tile:
    GpSimd: EQ[p,j,j'] = (x_j == x_j') (int8), strict-upper mask via
            affine_select, and the final per-partition local_scatter that
            writes the multiplicity T_j of each slot's value at column
            x_j for the LAST occurrence only (idx -1 elsewhere, skipped).
    Vector: the two add-reduces (T, ngt), the last-occurrence/index
            arithmetic, and the psum evacuations.
    PE:     chunkwise 128x128 transposes of C into a 4-tile group buffer
            C^T [128, nchunk, 512], then the W-stationary table matmul
            producing h^T [hid, b] directly (no second transpose), fc2,
            stacked cp/wdl heads, and the bucket block-sum selection.
    Scalar: the screlu squares (exact: PWP tables represent x^2).

  Only 5 DMA instructions are issued (2 packed const loads, x, pc, out);
  multi-sem waits are split by Bacc's generate_event_semaphores pass.
"""

import sys

sys.path.insert(0, "/opt/trn_rl_repo")

import numpy as np

import concourse.bacc as bacc
import concourse.mybir as mybir
from concourse import bass_utils
from concourse.bass import ds
from concourse.tile import TileContext

F16 = mybir.dt.float16
F32 = mybir.dt.float32
I16 = mybir.dt.int16
I8 = mybir.dt.int8
OP = mybir.AluOpType

INPUT_DIM = 768
HIDDEN = 1024
H2 = 32
PAD_IDX = 768
NCORES = 8
NSLOT = 32
GRP = 4  # 128-sample tiles per matmul group (512-wide moving side)

# small const layout (fp16, 32 partitions)
SM_HW = 0                  # [32, 32] head weightsT
SM_FC2B = 32
SM_HEADB = 33
SM_LO = 34
SM_HI = 35
SM_SEL = 36                # [32, 4]
SM_COLS = 40


def build_nc(BL, nchunk):
    """Build the Bass module for one core processing BL samples.

    nchunk: 6 when bias1 == 0 (vocab 768, pad index skipped), else 7
    (vocab 896 with bias1 in row 895 via a constant-1 column).
    """
    T = BL // 128
    if T >= 12 and T % 4 == 0:
        group_sizes = [2, 2] + [4] * ((T - 4) // 4)
    else:
        g0 = min(GRP, T)
        group_sizes = [g0] * (T // g0)
        assert T % g0 == 0
    GW = 128 * max(group_sizes)
    vocab = nchunk * 128
    with_bias = nchunk == 7
    nidx = NSLOT + 2 if with_bias else NSLOT

    # packed const layout (fp16, 128 partitions)
    off_w = 0                      # [128, nchunk*1024] table chunks
    off_fc2 = nchunk * HIDDEN      # [128, 8*32] fc2^T chunks
    wc_cols = off_fc2 + 8 * H2

    # Bacc (not plain Bass): its finalize() runs generate_event_semaphores,
    # which splits multi-sem waits down to the 1-wait-per-instruction TPB
    # encoding limit — walrus codegen rejects unsplit Tile output.
    nc = bacc.Bacc("TRN2", target_bir_lowering=False)

    wc_d = nc.declare_dram_parameter("wconst", [128, wc_cols], F16, isOutput=False)
    sm_d = nc.declare_dram_parameter("smconst", [32, SM_COLS], F16, isOutput=False)
    x_d = nc.declare_dram_parameter("x16", [128, T, NSLOT], F16, isOutput=False)
    pc_d = nc.declare_dram_parameter("pcrep", [32, BL], F16, isOutput=False)
    out_d = nc.declare_dram_parameter("out4", [4, BL], F32, isOutput=True)

    with TileContext(nc) as tc:
        with (
            tc.tile_pool(name="const", bufs=1) as cp,
            tc.tile_pool(name="work", bufs=3) as wp,
            tc.tile_pool(name="grp", bufs=3) as gp,
            tc.tile_pool(name="tail", bufs=2) as tp,
            tc.tile_pool(name="psh", bufs=3, space="PSUM") as psh,
            tc.tile_pool(name="pss", bufs=3, space="PSUM") as pss,
        ):
            # ---- inputs: x first (the count path only needs x), table last
            x_raw = cp.tile([128, T, NSLOT], F16)
            nc.sync.dma_start(out=x_raw, in_=x_d[:])
            sm_raw = cp.tile([32, SM_COLS], F16)
            nc.sync.dma_start(out=sm_raw, in_=sm_d[:])
            pc_raw = cp.tile([32, BL], F16)
            nc.sync.dma_start(out=pc_raw, in_=pc_d[:])
            wc_sb = cp.tile([128, wc_cols], F16)
            nc.sync.dma_start(out=wc_sb, in_=wc_d[:])

            # Vector-engine staging copies absorb DMA waits (TensorScalarPtr
            # ops encode at most one sync wait).
            x_sb = cp.tile([128, T, NSLOT], F16)
            nc.vector.tensor_copy(x_sb, x_raw)
            sm_sb = cp.tile([32, SM_COLS], F16)
            nc.vector.tensor_copy(sm_sb, sm_raw)
            pc_sb = cp.tile([32, BL], F16)
            nc.vector.tensor_copy(pc_sb, pc_raw)
            cons32 = cp.tile([32, 4], F32)
            nc.vector.tensor_copy(cons32, sm_sb[:, ds(SM_FC2B, 4)])

            final_sb = cp.tile([4, BL], F32)
            oh_sb = cp.tile([32, BL], F16)

            def build_oh():
                # bucket one-hot, whole batch: OH[p, b] = (bucket_b == p % 8)
                # v = ((pc-2)*8 + 0.5) / 30 ; bucket = clip(floor(v), 0, 7)
                v_sb = cp.tile([32, BL], F16)
                nc.vector.tensor_scalar(v_sb, pc_sb, 8.0 / 30.0, -15.5 / 30.0, OP.mult, OP.add)
                ge_sb = cp.tile([32, BL], F16)
                nc.vector.tensor_scalar(ge_sb, v_sb, cons32[:, ds(2, 1)], None, OP.is_ge)
                lt_sb = cp.tile([32, BL], F16)
                nc.vector.tensor_scalar(lt_sb, v_sb, cons32[:, ds(3, 1)], None, OP.is_lt)
                nc.vector.tensor_tensor(oh_sb, ge_sb, lt_sb, OP.mult)

            # pad mask over the whole batch in one op
            if not with_bias:
                padne_all = cp.tile([128, T, NSLOT], F16)
                nc.vector.tensor_scalar(
                    padne_all, x_sb, float(PAD_IDX), None, OP.not_equal
                )

            goff = 0
            for g, grp in enumerate(group_sizes):
                gw = 128 * grp
                gboff = 128 * goff  # batch offset of this group
                CT = gp.tile([128, nchunk, GW], F16, tag="CT")
                xs = x_sb[:, ds(goff, grp), :]

                # EQ[p, q, j, j'] = (x_j == x_j') for the whole group (Vector)
                EQ = wp.tile([128, grp, NSLOT, NSLOT], I8, tag="EQ")
                nc.vector.tensor_tensor(
                    EQ,
                    xs[:, :, :, None].broadcast_to([128, grp, NSLOT, NSLOT]),
                    xs[:, :, None, :].broadcast_to([128, grp, NSLOT, NSLOT]),
                    OP.is_equal,
                )
                # T_j = total multiplicity of x_j. Every slot scatters T at
                # column x_j; occurrences of the same value carry the same T,
                # so the HW scatter's last-write-wins duplicate handling
                # still yields column x = T. Pads map to idx -1 (skipped) in
                # the 6-chunk layout; with bias the pad column's W row is 0.
                data_f = wp.tile([128, grp, nidx], F16, tag="data")
                with nc.allow_low_precision(reason="counts <= 32 exact in fp16"):
                    nc.vector.tensor_reduce(
                        data_f[:, :, ds(0, NSLOT)], EQ, mybir.AxisListType.X, OP.add
                    )
                # idx = keep * (x + 1) - 1 ; keep kills pad indices
                idxm = wp.tile([128, grp, NSLOT], F16, tag="idxm")
                idxs_i = wp.tile([128, grp, nidx], I16, tag="idxs")
                if not with_bias:
                    nc.vector.scalar_tensor_tensor(
                        out=idxm, in0=xs, scalar=1.0,
                        in1=padne_all[:, ds(goff, grp), :],
                        op0=OP.add, op1=OP.mult,
                    )
                    nc.vector.tensor_scalar(
                        idxs_i[:, :, ds(0, NSLOT)], idxm, -1.0, None, OP.add
                    )
                else:
                    nc.vector.tensor_scalar(
                        idxs_i[:, :, ds(0, NSLOT)], xs, 0.0, None, OP.add
                    )
                    # fixed extra: column 895 <- 1.0 (bias1 row) + one pad
                    nc.vector.memset(idxs_i[:, :, ds(NSLOT, 1)], vocab - 1)
                    nc.vector.memset(idxs_i[:, :, ds(NSLOT + 1, 1)], -1)
                    nc.vector.memset(data_f[:, :, ds(NSLOT, 2)], 1.0)

                for ti in range(grp):
                    # C[b, v] (zeroed by the scatter itself)    (GpSimd)
                    C = wp.tile([128, vocab], F16, tag="C")
                    nc.gpsimd.local_scatter(
                        C, data_f[:, ti, :], idxs_i[:, ti, :],
                        channels=128, num_elems=vocab, num_idxs=nidx,
                    )
                    # xbar DMA transpose straight into the group buffer:
                    # CT[p, c, b] = C[b, c*128 + p]
                    nc.sync.dma_start_transpose(
                        out=CT[:, :, ds(128 * ti, 128)], in_=C[:]
                    )

                if g == 0:
                    build_oh()

                # ---- h^T = W_pad^T @ C^T, W-stationary, 512-wide moving side
                hq = gp.tile([128, 8, GW], F16, tag="hq")
                for ht in range(8):
                    ph = psh.tile([128, gw], F32, tag="h")
                    for c in range(nchunk):
                        nc.tensor.matmul(
                            ph[:],
                            wc_sb[:, ds(off_w + HIDDEN * c + 128 * ht, 128)],
                            CT[:, c, ds(0, gw)],
                            start=(c == 0),
                            stop=(c == nchunk - 1),
                        )
                    # screlu: relu on Scalar (psum read), min on Vector,
                    # square on Scalar — keeps the psum-evac pass off Vector
                    hr = wp.tile([128, gw], F16, tag="hr")
                    nc.scalar.activation(hr, ph, mybir.ActivationFunctionType.Relu)
                    hs = wp.tile([128, gw], F16, tag="hs")
                    nc.vector.tensor_scalar(hs, hr, 1.0, None, OP.min)
                    nc.scalar.square(out=hq[:, ht, ds(0, gw)], in_=hs)

                # ---- h2 = screlu(fc2T^T @ h^T + fc2_b) as [32, b]
                p2 = pss.tile([32, gw], F32, tag="small")
                for ht in range(8):
                    nc.tensor.matmul(
                        p2[:],
                        wc_sb[:, ds(off_fc2 + H2 * ht, H2)],
                        hq[:, ht, ds(0, gw)],
                        start=(ht == 0),
                        stop=(ht == 7),
                    )
                a2 = tp.tile([32, gw], F16, tag="a2")
                nc.vector.tensor_scalar(a2, p2, cons32[:, ds(0, 1)], 0.0, OP.add, OP.max)
                b2 = tp.tile([32, gw], F16, tag="b2")
                nc.vector.tensor_scalar(b2, a2, 1.0, None, OP.min)
                h2 = tp.tile([32, gw], F16, tag="h2")
                nc.scalar.square(out=h2, in_=b2)

                # ---- heads + bucket-select
                p3 = pss.tile([32, gw], F32, tag="small")
                nc.tensor.matmul(p3[:], sm_sb[:, ds(SM_HW, 32)], h2[:], start=True, stop=True)
                o4 = tp.tile([32, gw], F16, tag="o4")
                nc.vector.tensor_scalar(o4, p3, cons32[:, ds(1, 1)], None, OP.add)
                mk = tp.tile([32, gw], F16, tag="mk")
                nc.vector.tensor_tensor(mk, o4, oh_sb[:, ds(gboff, gw)], OP.mult)
                p4 = pss.tile([4, gw], F32, tag="small")
                nc.tensor.matmul(p4[:], sm_sb[:, ds(SM_SEL, 4)], mk[:], start=True, stop=True)
                nc.vector.tensor_copy(final_sb[:, ds(gboff, gw)], p4[:])
                nc.sync.dma_start(
                    out=out_d[:, ds(gboff, gw)], in_=final_sb[:, ds(gboff, gw)]
                )
                goff += grp


    nc.finalize()
    return nc


# ---------------------------------------------------------------------------
# host side


def _prep_shared(emb_w, bias1, fc2_w, fc2_b, cp_w, cp_b, wdl_w, wdl_b, nchunk):
    vocab = nchunk * 128
    emb = np.asarray(emb_w, np.float32).copy()
    emb[PAD_IDX] = 0.0
    wpad = np.zeros((vocab, HIDDEN), np.float32)
    wpad[: min(INPUT_DIM + 1, vocab)] = emb[: min(INPUT_DIM + 1, vocab)]
    if nchunk == 7:
        wpad[vocab - 1] = np.asarray(bias1, np.float32)
    wpad = wpad.reshape(nchunk, 128, HIDDEN).transpose(1, 0, 2)

    fc2t = np.asarray(fc2_w, np.float32).T  # [1024, 32]
    fc2t = fc2t.reshape(8, 128, H2).transpose(1, 0, 2)  # [128, 8, 32]

    off_w = 0
    off_fc2 = nchunk * HIDDEN
    wc_cols = off_fc2 + 8 * H2

    wconst = np.zeros((128, wc_cols), np.float16)
    wconst[:, off_w : off_w + nchunk * HIDDEN] = wpad.reshape(128, -1).astype(np.float16)
    wconst[:, off_fc2 : off_fc2 + 8 * H2] = fc2t.reshape(128, -1).astype(np.float16)

    stacked = np.zeros((32, H2), np.float32)
    stacked_b = np.zeros((32,), np.float32)
    stacked[0:8] = np.asarray(cp_w, np.float32)
    stacked_b[0:8] = np.asarray(cp_b, np.float32)
    for k in range(3):
        for u in range(8):
            stacked[8 + 8 * k + u] = np.asarray(wdl_w, np.float32)[3 * u + k]
            stacked_b[8 + 8 * k + u] = np.asarray(wdl_b, np.float32)[3 * u + k]

    smconst = np.zeros((32, SM_COLS), np.float16)
    smconst[:, SM_HW : SM_HW + 32] = stacked.T.astype(np.float16)  # [H2, 32]
    smconst[:, SM_FC2B] = np.asarray(fc2_b, np.float32).astype(np.float16)
    smconst[:, SM_HEADB] = stacked_b.astype(np.float16)
    uu = np.arange(32) % 8
    smconst[:, SM_LO] = np.where(uu == 0, -30000.0, uu).astype(np.float16)
    smconst[:, SM_HI] = np.where(uu == 7, 30000.0, uu + 1).astype(np.float16)
    sel = np.zeros((32, 4), np.float16)
    sel[np.arange(32), np.arange(32) // 8] = 1.0
    smconst[:, SM_SEL : SM_SEL + 4] = sel

    return dict(wconst=wconst, smconst=smconst)


def _prep_core(x_c, pc_c):
    BL = x_c.shape[0]
    T = BL // 128
    x16 = np.ascontiguousarray(
        np.asarray(x_c, np.int64)
        .astype(np.float16)
        .reshape(T, 128, NSLOT)
        .transpose(1, 0, 2)
    )
    pcrep = np.broadcast_to(
        np.asarray(pc_c, np.int64).astype(np.float16)[None, :], (32, BL)
    ).copy()
    return dict(x16=x16, pcrep=pcrep)


_NC_CACHE = {}


def kernel(x, piece_count, emb_w, bias1, fc2_w, fc2_b, cp_w, cp_b, wdl_w, wdl_b):
    x = np.asarray(x)
    piece_count = np.asarray(piece_count)
    B = x.shape[0]
    BL = B // NCORES
    nchunk = 6 if not np.any(np.asarray(bias1)) else 7

    key = (BL, nchunk)
    if key not in _NC_CACHE:
        _NC_CACHE[key] = build_nc(BL, nchunk)
    nc = _NC_CACHE[key]

    shared = _prep_shared(emb_w, bias1, fc2_w, fc2_b, cp_w, cp_b, wdl_w, wdl_b, nchunk)
    in_maps = []
    for c in range(NCORES):
        m = dict(shared)
        m.update(_prep_core(x[c * BL : (c + 1) * BL], piece_count[c * BL : (c + 1) * BL]))
        in_maps.append(m)

    res = bass_utils.run_bass_kernel_spmd(nc, in_maps, list(range(NCORES))).results
    out4 = np.concatenate([res[c]["out4"] for c in range(NCORES)], axis=1)  # [4, B]
    outT = out4.T.astype(np.float32)
    cp_out = np.ascontiguousarray(outT[:, 0:1])
    wdl_out = np.ascontiguousarray(outT[:, 1:4])
    return cp_out, wdl_out


# revision 29
# speedup vs baseline: 1.0095x; 1.0095x over previous
"""Trainium2 Bass kernel for EvalNet (EmbeddingBag + MLP + bucketed heads).

Strategy (pure data parallel over 8 cores, batch dim sharded):
  The EmbeddingBag-sum  h[b] = sum_j emb_w[x[b,j]]  is reformulated as a
  dense matmul  h^T = W_pad^T @ C^T  where C[b, v] counts how many of the
  32 slots of sample b hold index v.  Because emb_w's padding row (768)
  is zero and rows 769+ don't exist, the effective vocab is 768 = 6*128
  when bias1 == 0 (the usual case); with a nonzero bias1 a 7th chunk
  carries bias1 in row 895 and a constant-1 column is injected.

  Per 128-sample tile:
    GpSimd: EQ[p,j,j'] = (x_j == x_j') (int8), strict-upper mask via
            affine_select, and the final per-partition local_scatter that
            writes the multiplicity T_j of each slot's value at column
            x_j for the LAST occurrence only (idx -1 elsewhere, skipped).
    Vector: the two add-reduces (T, ngt), the last-occurrence/index
            arithmetic, and the psum evacuations.
    PE:     chunkwise 128x128 transposes of C into a 4-tile group buffer
            C^T [128, nchunk, 512], then the W-stationary table matmul
            producing h^T [hid, b] directly (no second transpose), fc2,
            stacked cp/wdl heads, and the bucket block-sum selection.
    Scalar: the screlu squares (exact: PWP tables represent x^2).

  Only 5 DMA instructions are issued (2 packed const loads, x, pc, out);
  multi-sem waits are split by Bacc's generate_event_semaphores pass.
"""

import sys

sys.path.insert(0, "/opt/trn_rl_repo")

import numpy as np

import concourse.bacc as bacc
import concourse.mybir as mybir
from concourse import bass_utils
from concourse.bass import ds
from concourse.tile import TileContext

F16 = mybir.dt.float16
F32 = mybir.dt.float32
I16 = mybir.dt.int16
I8 = mybir.dt.int8
OP = mybir.AluOpType

INPUT_DIM = 768
HIDDEN = 1024
H2 = 32
PAD_IDX = 768
NCORES = 8
NSLOT = 32
GRP = 4  # 128-sample tiles per matmul group (512-wide moving side)

# small const layout (fp16, 32 partitions)
SM_HW = 0                  # [32, 32] head weightsT
SM_FC2B = 32
SM_HEADB = 33
SM_LO = 34
SM_HI = 35
SM_SEL = 36                # [32, 4]
SM_COLS = 40


def build_nc(BL, nchunk):
    """Build the Bass module for one core processing BL samples.

    nchunk: 6 when bias1 == 0 (vocab 768, pad index skipped), else 7
    (vocab 896 with bias1 in row 895 via a constant-1 column).
    """
    T = BL // 128
    g0 = min(GRP, T)
    group_sizes = [g0] * (T // g0)
    assert T % g0 == 0
    GW = 128 * max(group_sizes)
    vocab = nchunk * 128
    with_bias = nchunk == 7
    nidx = NSLOT + 2 if with_bias else NSLOT

    # packed const layout (fp16, 128 partitions)
    off_w = 0                      # [128, nchunk*1024] table chunks
    off_fc2 = nchunk * HIDDEN      # [128, 8*32] fc2^T chunks
    wc_cols = off_fc2 + 8 * H2

    # Bacc (not plain Bass): its finalize() runs generate_event_semaphores,
    # which splits multi-sem waits down to the 1-wait-per-instruction TPB
    # encoding limit — walrus codegen rejects unsplit Tile output.
    nc = bacc.Bacc("TRN2", target_bir_lowering=False)

    wc_d = nc.declare_dram_parameter("wconst", [128, wc_cols], F16, isOutput=False)
    sm_d = nc.declare_dram_parameter("smconst", [32, SM_COLS], F16, isOutput=False)
    x_d = nc.declare_dram_parameter("x16", [128, T, NSLOT], F16, isOutput=False)
    pc_d = nc.declare_dram_parameter("pcrep", [32, BL], F16, isOutput=False)
    out_d = nc.declare_dram_parameter("out4", [4, BL], F32, isOutput=True)

    with TileContext(nc) as tc:
        with (
            tc.tile_pool(name="const", bufs=1) as cp,
            tc.tile_pool(name="work", bufs=3) as wp,
            tc.tile_pool(name="grp", bufs=4) as gp,
            tc.tile_pool(name="tail", bufs=2) as tp,
            tc.tile_pool(name="psh", bufs=3, space="PSUM") as psh,
            tc.tile_pool(name="pss", bufs=3, space="PSUM") as pss,
        ):
            # ---- inputs: x first (the count path only needs x), table last
            x_raw = cp.tile([128, T, NSLOT], F16)
            nc.sync.dma_start(out=x_raw, in_=x_d[:])
            sm_raw = cp.tile([32, SM_COLS], F16)
            nc.sync.dma_start(out=sm_raw, in_=sm_d[:])
            pc_raw = cp.tile([32, BL], F16)
            nc.sync.dma_start(out=pc_raw, in_=pc_d[:])
            wc_sb = cp.tile([128, wc_cols], F16)
            nc.sync.dma_start(out=wc_sb, in_=wc_d[:])

            # Vector-engine staging copies absorb DMA waits (TensorScalarPtr
            # ops encode at most one sync wait).
            x_sb = cp.tile([128, T, NSLOT], F16)
            nc.vector.tensor_copy(x_sb, x_raw)
            sm_sb = cp.tile([32, SM_COLS], F16)
            nc.vector.tensor_copy(sm_sb, sm_raw)
            pc_sb = cp.tile([32, BL], F16)
            nc.vector.tensor_copy(pc_sb, pc_raw)
            cons32 = cp.tile([32, 4], F32)
            nc.vector.tensor_copy(cons32, sm_sb[:, ds(SM_FC2B, 4)])

            final_sb = cp.tile([4, BL], F32)
            oh_sb = cp.tile([32, BL], F16)

            def build_oh():
                # bucket one-hot, whole batch: OH[p, b] = (bucket_b == p % 8)
                # v = ((pc-2)*8 + 0.5) / 30 ; bucket = clip(floor(v), 0, 7)
                v_sb = cp.tile([32, BL], F16)
                nc.vector.tensor_scalar(v_sb, pc_sb, 8.0 / 30.0, -15.5 / 30.0, OP.mult, OP.add)
                ge_sb = cp.tile([32, BL], F16)
                nc.vector.tensor_scalar(ge_sb, v_sb, cons32[:, ds(2, 1)], None, OP.is_ge)
                lt_sb = cp.tile([32, BL], F16)
                nc.vector.tensor_scalar(lt_sb, v_sb, cons32[:, ds(3, 1)], None, OP.is_lt)
                nc.vector.tensor_tensor(oh_sb, ge_sb, lt_sb, OP.mult)

            # pad mask over the whole batch in one op
            if not with_bias:
                padne_all = cp.tile([128, T, NSLOT], F16)
                nc.vector.tensor_scalar(
                    padne_all, x_sb, float(PAD_IDX), None, OP.not_equal
                )

            CTs = []
            goff = 0
            for g, grp in enumerate(group_sizes):
                gw = 128 * grp
                CT = gp.tile([128, nchunk, GW], F16, tag="CT")
                CTs.append(CT)
                xs = x_sb[:, ds(goff, grp), :]

                # EQ[p, q, j, j'] = (x_j == x_j') for the whole group (Vector)
                EQ = wp.tile([128, grp, NSLOT, NSLOT], I8, tag="EQ")
                nc.vector.tensor_tensor(
                    EQ,
                    xs[:, :, :, None].broadcast_to([128, grp, NSLOT, NSLOT]),
                    xs[:, :, None, :].broadcast_to([128, grp, NSLOT, NSLOT]),
                    OP.is_equal,
                )
                # T_j = total multiplicity of x_j. Every slot scatters T at
                # column x_j; occurrences of the same value carry the same T,
                # so the HW scatter's last-write-wins duplicate handling
                # still yields column x = T. Pads map to idx -1 (skipped) in
                # the 6-chunk layout; with bias the pad column's W row is 0.
                data_f = wp.tile([128, grp, nidx], F16, tag="data")
                with nc.allow_low_precision(reason="counts <= 32 exact in fp16"):
                    nc.vector.tensor_reduce(
                        data_f[:, :, ds(0, NSLOT)], EQ, mybir.AxisListType.X, OP.add
                    )
                # idx = keep * (x + 1) - 1 ; keep kills pad indices
                idxm = wp.tile([128, grp, NSLOT], F16, tag="idxm")
                idxs_i = wp.tile([128, grp, nidx], I16, tag="idxs")
                if not with_bias:
                    nc.vector.scalar_tensor_tensor(
                        out=idxm, in0=xs, scalar=1.0,
                        in1=padne_all[:, ds(goff, grp), :],
                        op0=OP.add, op1=OP.mult,
                    )
                    nc.vector.tensor_scalar(
                        idxs_i[:, :, ds(0, NSLOT)], idxm, -1.0, None, OP.add
                    )
                else:
                    nc.vector.tensor_scalar(
                        idxs_i[:, :, ds(0, NSLOT)], xs, 0.0, None, OP.add
                    )
                    # fixed extra: column 895 <- 1.0 (bias1 row) + one pad
                    nc.vector.memset(idxs_i[:, :, ds(NSLOT, 1)], vocab - 1)
                    nc.vector.memset(idxs_i[:, :, ds(NSLOT + 1, 1)], -1)
                    nc.vector.memset(data_f[:, :, ds(NSLOT, 2)], 1.0)

                for ti in range(grp):
                    # C[b, v] (zeroed by the scatter itself)    (GpSimd)
                    C = wp.tile([128, vocab], F16, tag="C")
                    nc.gpsimd.local_scatter(
                        C, data_f[:, ti, :], idxs_i[:, ti, :],
                        channels=128, num_elems=vocab, num_idxs=nidx,
                    )
                    # xbar DMA transpose straight into the group buffer:
                    # CT[p, c, b] = C[b, c*128 + p]
                    nc.sync.dma_start_transpose(
                        out=CT[:, :, ds(128 * ti, 128)], in_=C[:]
                    )

                goff += grp

            goff = 0
            for g, grp in enumerate(group_sizes):
                gw = 128 * grp
                gboff = 128 * goff  # batch offset of this group
                CT = CTs[g]
                if g == 0:
                    build_oh()

                # ---- h^T = W_pad^T @ C^T, W-stationary, 512-wide moving side
                hq = gp.tile([128, 8, GW], F16, tag="hq")
                for ht in range(8):
                    ph = psh.tile([128, gw], F32, tag="h")
                    for c in range(nchunk):
                        nc.tensor.matmul(
                            ph[:],
                            wc_sb[:, ds(off_w + HIDDEN * c + 128 * ht, 128)],
                            CT[:, c, ds(0, gw)],
                            start=(c == 0),
                            stop=(c == nchunk - 1),
                        )
                    # screlu: relu on Scalar (psum read), min on Vector,
                    # square on Scalar — keeps the psum-evac pass off Vector
                    hr = wp.tile([128, gw], F16, tag="hr")
                    nc.scalar.activation(hr, ph, mybir.ActivationFunctionType.Relu)
                    hs = wp.tile([128, gw], F16, tag="hs")
                    nc.vector.tensor_scalar(hs, hr, 1.0, None, OP.min)
                    nc.scalar.square(out=hq[:, ht, ds(0, gw)], in_=hs)

                # ---- h2 = screlu(fc2T^T @ h^T + fc2_b) as [32, b]
                p2 = pss.tile([32, gw], F32, tag="small")
                for ht in range(8):
                    nc.tensor.matmul(
                        p2[:],
                        wc_sb[:, ds(off_fc2 + H2 * ht, H2)],
                        hq[:, ht, ds(0, gw)],
                        start=(ht == 0),
                        stop=(ht == 7),
                    )
                a2 = tp.tile([32, gw], F16, tag="a2")
                nc.vector.tensor_scalar(a2, p2, cons32[:, ds(0, 1)], 0.0, OP.add, OP.max)
                b2 = tp.tile([32, gw], F16, tag="b2")
                nc.vector.tensor_scalar(b2, a2, 1.0, None, OP.min)
                h2 = tp.tile([32, gw], F16, tag="h2")
                nc.scalar.square(out=h2, in_=b2)

                # ---- heads + bucket-select
                p3 = pss.tile([32, gw], F32, tag="small")
                nc.tensor.matmul(p3[:], sm_sb[:, ds(SM_HW, 32)], h2[:], start=True, stop=True)
                o4 = tp.tile([32, gw], F16, tag="o4")
                nc.vector.tensor_scalar(o4, p3, cons32[:, ds(1, 1)], None, OP.add)
                mk = tp.tile([32, gw], F16, tag="mk")
                nc.vector.tensor_tensor(mk, o4, oh_sb[:, ds(gboff, gw)], OP.mult)
                p4 = pss.tile([4, gw], F32, tag="small")
                nc.tensor.matmul(p4[:], sm_sb[:, ds(SM_SEL, 4)], mk[:], start=True, stop=True)
                nc.vector.tensor_copy(final_sb[:, ds(gboff, gw)], p4[:])
                nc.sync.dma_start(
                    out=out_d[:, ds(gboff, gw)], in_=final_sb[:, ds(gboff, gw)]
                )
                goff += grp


    nc.finalize()
    return nc


# ---------------------------------------------------------------------------
# host side


def _prep_shared(emb_w, bias1, fc2_w, fc2_b, cp_w, cp_b, wdl_w, wdl_b, nchunk):
    vocab = nchunk * 128
    emb = np.asarray(emb_w, np.float32).copy()
    emb[PAD_IDX] = 0.0
    wpad = np.zeros((vocab, HIDDEN), np.float32)
    wpad[: min(INPUT_DIM + 1, vocab)] = emb[: min(INPUT_DIM + 1, vocab)]
    if nchunk == 7:
        wpad[vocab - 1] = np.asarray(bias1, np.float32)
    wpad = wpad.reshape(nchunk, 128, HIDDEN).transpose(1, 0, 2)

    fc2t = np.asarray(fc2_w, np.float32).T  # [1024, 32]
    fc2t = fc2t.reshape(8, 128, H2).transpose(1, 0, 2)  # [128, 8, 32]

    off_w = 0
    off_fc2 = nchunk * HIDDEN
    wc_cols = off_fc2 + 8 * H2

    wconst = np.zeros((128, wc_cols), np.float16)
    wconst[:, off_w : off_w + nchunk * HIDDEN] = wpad.reshape(128, -1).astype(np.float16)
    wconst[:, off_fc2 : off_fc2 + 8 * H2] = fc2t.reshape(128, -1).astype(np.float16)

    stacked = np.zeros((32, H2), np.float32)
    stacked_b = np.zeros((32,), np.float32)
    stacked[0:8] = np.asarray(cp_w, np.float32)
    stacked_b[0:8] = np.asarray(cp_b, np.float32)
    for k in range(3):
        for u in range(8):
            stacked[8 + 8 * k + u] = np.asarray(wdl_w, np.float32)[3 * u + k]
            stacked_b[8 + 8 * k + u] = np.asarray(wdl_b, np.float32)[3 * u + k]

    smconst = np.zeros((32, SM_COLS), np.float16)
    smconst[:, SM_HW : SM_HW + 32] = stacked.T.astype(np.float16)  # [H2, 32]
    smconst[:, SM_FC2B] = np.asarray(fc2_b, np.float32).astype(np.float16)
    smconst[:, SM_HEADB] = stacked_b.astype(np.float16)
    uu = np.arange(32) % 8
    smconst[:, SM_LO] = np.where(uu == 0, -30000.0, uu).astype(np.float16)
    smconst[:, SM_HI] = np.where(uu == 7, 30000.0, uu + 1).astype(np.float16)
    sel = np.zeros((32, 4), np.float16)
    sel[np.arange(32), np.arange(32) // 8] = 1.0
    smconst[:, SM_SEL : SM_SEL + 4] = sel

    return dict(wconst=wconst, smconst=smconst)


def _prep_core(x_c, pc_c):
    BL = x_c.shape[0]
    T = BL // 128
    x16 = np.ascontiguousarray(
        np.asarray(x_c, np.int64)
        .astype(np.float16)
        .reshape(T, 128, NSLOT)
        .transpose(1, 0, 2)
    )
    pcrep = np.broadcast_to(
        np.asarray(pc_c, np.int64).astype(np.float16)[None, :], (32, BL)
    ).copy()
    return dict(x16=x16, pcrep=pcrep)


_NC_CACHE = {}


def kernel(x, piece_count, emb_w, bias1, fc2_w, fc2_b, cp_w, cp_b, wdl_w, wdl_b):
    x = np.asarray(x)
    piece_count = np.asarray(piece_count)
    B = x.shape[0]
    BL = B // NCORES
    nchunk = 6 if not np.any(np.asarray(bias1)) else 7

    key = (BL, nchunk)
    if key not in _NC_CACHE:
        _NC_CACHE[key] = build_nc(BL, nchunk)
    nc = _NC_CACHE[key]

    shared = _prep_shared(emb_w, bias1, fc2_w, fc2_b, cp_w, cp_b, wdl_w, wdl_b, nchunk)
    in_maps = []
    for c in range(NCORES):
        m = dict(shared)
        m.update(_prep_core(x[c * BL : (c + 1) * BL], piece_count[c * BL : (c + 1) * BL]))
        in_maps.append(m)

    res = bass_utils.run_bass_kernel_spmd(nc, in_maps, list(range(NCORES))).results
    out4 = np.concatenate([res[c]["out4"] for c in range(NCORES)], axis=1)  # [4, B]
    outT = out4.T.astype(np.float32)
    cp_out = np.ascontiguousarray(outT[:, 0:1])
    wdl_out = np.ascontiguousarray(outT[:, 1:4])
    return cp_out, wdl_out


# revision 30
# speedup vs baseline: 1.0240x; 1.0143x over previous
"""Trainium2 Bass kernel for EvalNet (EmbeddingBag + MLP + bucketed heads).

Strategy (pure data parallel over 8 cores, batch dim sharded):
  The EmbeddingBag-sum  h[b] = sum_j emb_w[x[b,j]]  is reformulated as a
  dense matmul  h^T = W_pad^T @ C^T  where C[b, v] counts how many of the
  32 slots of sample b hold index v.  Because emb_w's padding row (768)
  is zero and rows 769+ don't exist, the effective vocab is 768 = 6*128
  when bias1 == 0 (the usual case); with a nonzero bias1 a 7th chunk
  carries bias1 in row 895 and a constant-1 column is injected.

  Per 128-sample tile:
    GpSimd: EQ[p,j,j'] = (x_j == x_j') (int8), strict-upper mask via
            affine_select, and the final per-partition local_scatter that
            writes the multiplicity T_j of each slot's value at column
            x_j for the LAST occurrence only (idx -1 elsewhere, skipped).
    Vector: the two add-reduces (T, ngt), the last-occurrence/index
            arithmetic, and the psum evacuations.
    PE:     chunkwise 128x128 transposes of C into a 4-tile group buffer
            C^T [128, nchunk, 512], then the W-stationary table matmul
            producing h^T [hid, b] directly (no second transpose), fc2,
            stacked cp/wdl heads, and the bucket block-sum selection.
    Scalar: the screlu squares (exact: PWP tables represent x^2).

  Only 5 DMA instructions are issued (2 packed const loads, x, pc, out);
  multi-sem waits are split by Bacc's generate_event_semaphores pass.
"""

import sys

sys.path.insert(0, "/opt/trn_rl_repo")

import numpy as np

import concourse.bacc as bacc
import concourse.mybir as mybir
from concourse import bass_utils
from concourse.bass import ds
from concourse.tile import TileContext

F16 = mybir.dt.float16
F32 = mybir.dt.float32
I16 = mybir.dt.int16
I8 = mybir.dt.int8
OP = mybir.AluOpType

INPUT_DIM = 768
HIDDEN = 1024
H2 = 32
PAD_IDX = 768
NCORES = 8
NSLOT = 32
GRP = 4  # 128-sample tiles per matmul group (512-wide moving side)

# small const layout (fp16, 32 partitions)
SM_HW = 0                  # [32, 32] head weightsT
SM_FC2B = 32
SM_HEADB = 33
SM_LO = 34
SM_HI = 35
SM_SEL = 36                # [32, 4]
SM_COLS = 40


def build_nc(BL, nchunk):
    """Build the Bass module for one core processing BL samples.

    nchunk: 6 when bias1 == 0 (vocab 768, pad index skipped), else 7
    (vocab 896 with bias1 in row 895 via a constant-1 column).
    """
    T = BL // 128
    g0 = min(GRP, T)
    group_sizes = [g0] * (T // g0)
    assert T % g0 == 0
    GW = 128 * max(group_sizes)
    vocab = nchunk * 128
    with_bias = nchunk == 7
    nidx = NSLOT + 2 if with_bias else NSLOT

    # packed const layout (fp16, 128 partitions)
    off_w = 0                      # [128, nchunk*1024] table chunks
    off_fc2 = nchunk * HIDDEN      # [128, 8*32] fc2^T chunks
    wc_cols = off_fc2 + 8 * H2

    # Bacc (not plain Bass): its finalize() runs generate_event_semaphores,
    # which splits multi-sem waits down to the 1-wait-per-instruction TPB
    # encoding limit — walrus codegen rejects unsplit Tile output.
    nc = bacc.Bacc("TRN2", target_bir_lowering=False)

    wc_d = nc.declare_dram_parameter("wconst", [128, wc_cols], F16, isOutput=False)
    sm_d = nc.declare_dram_parameter("smconst", [32, SM_COLS], F16, isOutput=False)
    x_d = nc.declare_dram_parameter("x16", [128, T, NSLOT], F16, isOutput=False)
    pc_d = nc.declare_dram_parameter("pcrep", [32, BL], F16, isOutput=False)
    out_d = nc.declare_dram_parameter("out4", [4, BL], F32, isOutput=True)

    with TileContext(nc) as tc:
        with (
            tc.tile_pool(name="const", bufs=1) as cp,
            tc.tile_pool(name="work", bufs=3) as wp,
            tc.tile_pool(name="grp", bufs=4) as gp,
            tc.tile_pool(name="tail", bufs=2) as tp,
            tc.tile_pool(name="psh", bufs=3, space="PSUM") as psh,
            tc.tile_pool(name="pss", bufs=3, space="PSUM") as pss,
        ):
            # ---- inputs: x first (the count path only needs x), table last
            x_raw = cp.tile([128, T, NSLOT], F16)
            nc.sync.dma_start(out=x_raw, in_=x_d[:])
            sm_raw = cp.tile([32, SM_COLS], F16)
            nc.sync.dma_start(out=sm_raw, in_=sm_d[:])
            pc_raw = cp.tile([32, BL], F16)
            nc.sync.dma_start(out=pc_raw, in_=pc_d[:])
            wc_sb = cp.tile([128, wc_cols], F16)
            nc.sync.dma_start(out=wc_sb, in_=wc_d[:])

            # Vector-engine staging copies absorb DMA waits for the
            # TensorScalarPtr consumers (<=1 wait in their encoding).
            sm_sb = cp.tile([32, SM_COLS], F16)
            nc.vector.tensor_copy(sm_sb, sm_raw)
            pc_sb = cp.tile([32, BL], F16)
            nc.vector.tensor_copy(pc_sb, pc_raw)
            cons32 = cp.tile([32, 4], F32)
            nc.vector.tensor_copy(cons32, sm_sb[:, ds(SM_FC2B, 4)])

            final_sb = cp.tile([4, BL], F32)
            oh_sb = cp.tile([32, BL], F16)

            def build_oh():
                # bucket one-hot, whole batch: OH[p, b] = (bucket_b == p % 8)
                # v = ((pc-2)*8 + 0.5) / 30 ; bucket = clip(floor(v), 0, 7)
                v_sb = cp.tile([32, BL], F16)
                nc.vector.tensor_scalar(v_sb, pc_sb, 8.0 / 30.0, -15.5 / 30.0, OP.mult, OP.add)
                ge_sb = cp.tile([32, BL], F16)
                nc.vector.tensor_scalar(ge_sb, v_sb, cons32[:, ds(2, 1)], None, OP.is_ge)
                lt_sb = cp.tile([32, BL], F16)
                nc.vector.tensor_scalar(lt_sb, v_sb, cons32[:, ds(3, 1)], None, OP.is_lt)
                nc.vector.tensor_tensor(oh_sb, ge_sb, lt_sb, OP.mult)

            # pad mask over the whole batch in one op
            if not with_bias:
                padne_all = cp.tile([128, T, NSLOT], F16)
                nc.vector.tensor_scalar(
                    padne_all, x_raw, float(PAD_IDX), None, OP.not_equal
                )

            def count_path(tiles, toff):
                """Count-path for `tiles` consecutive 128-sample tiles starting
                at tile `toff`: EQ -> T -> idx -> scatter -> xbar transpose.
                Returns the per-tile C tiles' transposes already issued into
                the CT tile given. T_j (total multiplicity of x_j) is
                scattered at column x_j for EVERY slot; equal-valued slots
                carry equal T, so the scatter's last-write-wins duplicate
                handling still ends at column x = T. Pads map to idx -1
                (skipped) in the 6-chunk layout; with bias the pad column's
                W row is 0."""
                xs = x_raw[:, ds(toff, tiles), :]
                EQ = wp.tile([128, tiles, NSLOT, NSLOT], I8, tag="EQ")
                nc.vector.tensor_tensor(
                    EQ,
                    xs[:, :, :, None].broadcast_to([128, tiles, NSLOT, NSLOT]),
                    xs[:, :, None, :].broadcast_to([128, tiles, NSLOT, NSLOT]),
                    OP.is_equal,
                )
                data_f = wp.tile([128, tiles, nidx], F16, tag="data")
                with nc.allow_low_precision(reason="counts <= 32 exact in fp16"):
                    nc.vector.tensor_reduce(
                        data_f[:, :, ds(0, NSLOT)], EQ, mybir.AxisListType.X, OP.add
                    )
                idxs_i = wp.tile([128, tiles, nidx], I16, tag="idxs")
                if not with_bias:
                    idxm = wp.tile([128, tiles, NSLOT], F16, tag="idxm")
                    nc.vector.scalar_tensor_tensor(
                        out=idxm, in0=xs, scalar=1.0,
                        in1=padne_all[:, ds(toff, tiles), :],
                        op0=OP.add, op1=OP.mult,
                    )
                    nc.vector.tensor_scalar(
                        idxs_i[:, :, ds(0, NSLOT)], idxm, -1.0, None, OP.add
                    )
                else:
                    nc.vector.tensor_scalar(
                        idxs_i[:, :, ds(0, NSLOT)], xs, 0.0, None, OP.add
                    )
                    nc.vector.memset(idxs_i[:, :, ds(NSLOT, 1)], vocab - 1)
                    nc.vector.memset(idxs_i[:, :, ds(NSLOT + 1, 1)], -1)
                    nc.vector.memset(data_f[:, :, ds(NSLOT, 2)], 1.0)
                return data_f, idxs_i

            def scatter_transpose(CT, data_f, idxs_i, ti, cti):
                C = wp.tile([128, vocab], F16, tag="C")
                nc.gpsimd.local_scatter(
                    C, data_f[:, ti, :], idxs_i[:, ti, :],
                    channels=128, num_elems=vocab, num_idxs=nidx,
                )
                # xbar DMA transpose: CT[p, c, b] = C[b, c*128 + p]
                nc.sync.dma_start_transpose(
                    out=CT[:, :, ds(128 * cti, 128)], in_=C[:]
                )

            for g, grp in enumerate(group_sizes):
                gw = 128 * grp
                goff = g * grp
                gboff = 128 * goff
                CT = gp.tile([128, nchunk, GW], F16, tag="CT")
                if g == 0:
                    # single-tile count paths: the first C^T slice (and hence
                    # the first matmul) is ready as early as possible
                    for ti in range(grp):
                        d1, i1 = count_path(1, ti)
                        scatter_transpose(CT, d1, i1, 0, ti)
                    build_oh()
                else:
                    data_f, idxs_i = count_path(grp, goff)
                    for ti in range(grp):
                        scatter_transpose(CT, data_f, idxs_i, ti, ti)

                # ---- h^T = W_pad^T @ C^T, W-stationary, 512-wide moving side
                hq = gp.tile([128, 8, GW], F16, tag="hq")
                for ht in range(8):
                    ph = psh.tile([128, gw], F32, tag="h")
                    for c in range(nchunk):
                        nc.tensor.matmul(
                            ph[:],
                            wc_sb[:, ds(off_w + HIDDEN * c + 128 * ht, 128)],
                            CT[:, c, ds(0, gw)],
                            start=(c == 0),
                            stop=(c == nchunk - 1),
                        )
                    # screlu: relu on Scalar (psum read), min on Vector,
                    # square on Scalar — keeps the psum-evac pass off Vector
                    hr = wp.tile([128, gw], F16, tag="hr")
                    nc.scalar.activation(hr, ph, mybir.ActivationFunctionType.Relu)
                    hs = wp.tile([128, gw], F16, tag="hs")
                    nc.vector.tensor_scalar(hs, hr, 1.0, None, OP.min)
                    nc.scalar.square(out=hq[:, ht, ds(0, gw)], in_=hs)

                # ---- h2 = screlu(fc2T^T @ h^T + fc2_b) as [32, b]
                p2 = pss.tile([32, gw], F32, tag="small")
                for ht in range(8):
                    nc.tensor.matmul(
                        p2[:],
                        wc_sb[:, ds(off_fc2 + H2 * ht, H2)],
                        hq[:, ht, ds(0, gw)],
                        start=(ht == 0),
                        stop=(ht == 7),
                    )
                a2 = tp.tile([32, gw], F16, tag="a2")
                nc.vector.tensor_scalar(a2, p2, cons32[:, ds(0, 1)], 0.0, OP.add, OP.max)
                b2 = tp.tile([32, gw], F16, tag="b2")
                nc.vector.tensor_scalar(b2, a2, 1.0, None, OP.min)
                h2 = tp.tile([32, gw], F16, tag="h2")
                nc.scalar.square(out=h2, in_=b2)

                # ---- heads + bucket-select
                p3 = pss.tile([32, gw], F32, tag="small")
                nc.tensor.matmul(p3[:], sm_sb[:, ds(SM_HW, 32)], h2[:], start=True, stop=True)
                o4 = tp.tile([32, gw], F16, tag="o4")
                nc.vector.tensor_scalar(o4, p3, cons32[:, ds(1, 1)], None, OP.add)
                mk = tp.tile([32, gw], F16, tag="mk")
                nc.vector.tensor_tensor(mk, o4, oh_sb[:, ds(gboff, gw)], OP.mult)
                p4 = pss.tile([4, gw], F32, tag="small")
                nc.tensor.matmul(p4[:], sm_sb[:, ds(SM_SEL, 4)], mk[:], start=True, stop=True)
                nc.vector.tensor_copy(final_sb[:, ds(gboff, gw)], p4[:])
                nc.sync.dma_start(
                    out=out_d[:, ds(gboff, gw)], in_=final_sb[:, ds(gboff, gw)]
                )

    nc.finalize()
    return nc


# ---------------------------------------------------------------------------
# host side


def _prep_shared(emb_w, bias1, fc2_w, fc2_b, cp_w, cp_b, wdl_w, wdl_b, nchunk):
    vocab = nchunk * 128
    emb = np.asarray(emb_w, np.float32).copy()
    emb[PAD_IDX] = 0.0
    wpad = np.zeros((vocab, HIDDEN), np.float32)
    wpad[: min(INPUT_DIM + 1, vocab)] = emb[: min(INPUT_DIM + 1, vocab)]
    if nchunk == 7:
        wpad[vocab - 1] = np.asarray(bias1, np.float32)
    wpad = wpad.reshape(nchunk, 128, HIDDEN).transpose(1, 0, 2)

    fc2t = np.asarray(fc2_w, np.float32).T  # [1024, 32]
    fc2t = fc2t.reshape(8, 128, H2).transpose(1, 0, 2)  # [128, 8, 32]

    off_w = 0
    off_fc2 = nchunk * HIDDEN
    wc_cols = off_fc2 + 8 * H2

    wconst = np.zeros((128, wc_cols), np.float16)
    wconst[:, off_w : off_w + nchunk * HIDDEN] = wpad.reshape(128, -1).astype(np.float16)
    wconst[:, off_fc2 : off_fc2 + 8 * H2] = fc2t.reshape(128, -1).astype(np.float16)

    stacked = np.zeros((32, H2), np.float32)
    stacked_b = np.zeros((32,), np.float32)
    stacked[0:8] = np.asarray(cp_w, np.float32)
    stacked_b[0:8] = np.asarray(cp_b, np.float32)
    for k in range(3):
        for u in range(8):
            stacked[8 + 8 * k + u] = np.asarray(wdl_w, np.float32)[3 * u + k]
            stacked_b[8 + 8 * k + u] = np.asarray(wdl_b, np.float32)[3 * u + k]

    smconst = np.zeros((32, SM_COLS), np.float16)
    smconst[:, SM_HW : SM_HW + 32] = stacked.T.astype(np.float16)  # [H2, 32]
    smconst[:, SM_FC2B] = np.asarray(fc2_b, np.float32).astype(np.float16)
    smconst[:, SM_HEADB] = stacked_b.astype(np.float16)
    uu = np.arange(32) % 8
    smconst[:, SM_LO] = np.where(uu == 0, -30000.0, uu).astype(np.float16)
    smconst[:, SM_HI] = np.where(uu == 7, 30000.0, uu + 1).astype(np.float16)
    sel = np.zeros((32, 4), np.float16)
    sel[np.arange(32), np.arange(32) // 8] = 1.0
    smconst[:, SM_SEL : SM_SEL + 4] = sel

    return dict(wconst=wconst, smconst=smconst)


def _prep_core(x_c, pc_c):
    BL = x_c.shape[0]
    T = BL // 128
    x16 = np.ascontiguousarray(
        np.asarray(x_c, np.int64)
        .astype(np.float16)
        .reshape(T, 128, NSLOT)
        .transpose(1, 0, 2)
    )
    pcrep = np.broadcast_to(
        np.asarray(pc_c, np.int64).astype(np.float16)[None, :], (32, BL)
    ).copy()
    return dict(x16=x16, pcrep=pcrep)


_NC_CACHE = {}


def kernel(x, piece_count, emb_w, bias1, fc2_w, fc2_b, cp_w, cp_b, wdl_w, wdl_b):
    x = np.asarray(x)
    piece_count = np.asarray(piece_count)
    B = x.shape[0]
    BL = B // NCORES
    nchunk = 6 if not np.any(np.asarray(bias1)) else 7

    key = (BL, nchunk)
    if key not in _NC_CACHE:
        _NC_CACHE[key] = build_nc(BL, nchunk)
    nc = _NC_CACHE[key]

    shared = _prep_shared(emb_w, bias1, fc2_w, fc2_b, cp_w, cp_b, wdl_w, wdl_b, nchunk)
    in_maps = []
    for c in range(NCORES):
        m = dict(shared)
        m.update(_prep_core(x[c * BL : (c + 1) * BL], piece_count[c * BL : (c + 1) * BL]))
        in_maps.append(m)

    res = bass_utils.run_bass_kernel_spmd(nc, in_maps, list(range(NCORES))).results
    out4 = np.concatenate([res[c]["out4"] for c in range(NCORES)], axis=1)  # [4, B]
    outT = out4.T.astype(np.float32)
    cp_out = np.ascontiguousarray(outT[:, 0:1])
    wdl_out = np.ascontiguousarray(outT[:, 1:4])
    return cp_out, wdl_out


# revision 31
# speedup vs baseline: 1.1046x; 1.0787x over previous
"""Trainium2 Bass kernel for EvalNet (EmbeddingBag + MLP + bucketed heads).

Strategy (pure data parallel over 8 cores, batch dim sharded):
  The EmbeddingBag-sum  h[b] = sum_j emb_w[x[b,j]]  is reformulated as a
  dense matmul  h^T = W_pad^T @ C^T  where C[b, v] counts how many of the
  32 slots of sample b hold index v.  Because emb_w's padding row (768)
  is zero and rows 769+ don't exist, the effective vocab is 768 = 6*128
  when bias1 == 0 (the usual case); with a nonzero bias1 a 7th chunk
  carries bias1 in row 895 and a constant-1 column is injected.

  Per 128-sample tile:
    GpSimd: EQ[p,j,j'] = (x_j == x_j') (int8), strict-upper mask via
            affine_select, and the final per-partition local_scatter that
            writes the multiplicity T_j of each slot's value at column
            x_j for the LAST occurrence only (idx -1 elsewhere, skipped).
    Vector: the two add-reduces (T, ngt), the last-occurrence/index
            arithmetic, and the psum evacuations.
    PE:     chunkwise 128x128 transposes of C into a 4-tile group buffer
            C^T [128, nchunk, 512], then the W-stationary table matmul
            producing h^T [hid, b] directly (no second transpose), fc2,
            stacked cp/wdl heads, and the bucket block-sum selection.
    Scalar: the screlu squares (exact: PWP tables represent x^2).

  Only 5 DMA instructions are issued (2 packed const loads, x, pc, out);
  multi-sem waits are split by Bacc's generate_event_semaphores pass.
"""

import sys

sys.path.insert(0, "/opt/trn_rl_repo")

import numpy as np

import concourse.bacc as bacc
import concourse.mybir as mybir
from concourse import bass_utils
from concourse.bass import ds
from concourse.tile import TileContext

F16 = mybir.dt.float16
F32 = mybir.dt.float32
I16 = mybir.dt.int16
I8 = mybir.dt.int8
OP = mybir.AluOpType

INPUT_DIM = 768
HIDDEN = 1024
H2 = 32
PAD_IDX = 768
NCORES = 8
NSLOT = 32
GRP = 4  # 128-sample tiles per matmul group (512-wide moving side)

# small const layout (fp16, 32 partitions)
SM_HW = 0                  # [32, 32] head weightsT
SM_FC2B = 32
SM_HEADB = 33
SM_LO = 34
SM_HI = 35
SM_SEL = 36                # [32, 4]
SM_COLS = 40


def build_nc(BL, nchunk):
    """Build the Bass module for one core processing BL samples.

    nchunk: 6 when bias1 == 0 (vocab 768, pad index skipped), else 7
    (vocab 896 with bias1 in row 895 via a constant-1 column).
    """
    T = BL // 128
    g0 = min(GRP, T)
    group_sizes = [g0] * (T // g0)
    assert T % g0 == 0
    GW = 128 * max(group_sizes)
    vocab = nchunk * 128
    with_bias = nchunk == 7
    nidx = NSLOT + 2 if with_bias else NSLOT

    # packed const layout (fp16, 128 partitions)
    off_w = 0                      # [128, nchunk*1024] table chunks
    off_fc2 = nchunk * HIDDEN      # [128, 8*32] fc2^T chunks
    wc_cols = off_fc2 + 8 * H2

    # Bacc (not plain Bass): its finalize() runs generate_event_semaphores,
    # which splits multi-sem waits down to the 1-wait-per-instruction TPB
    # encoding limit — walrus codegen rejects unsplit Tile output.
    nc = bacc.Bacc("TRN2", target_bir_lowering=False)

    wc_d = nc.declare_dram_parameter("wconst", [128, wc_cols], F16, isOutput=False)
    sm_d = nc.declare_dram_parameter("smconst", [32, SM_COLS], F16, isOutput=False)
    x_d = nc.declare_dram_parameter("x16", [128, T, NSLOT], F16, isOutput=False)
    pc_d = nc.declare_dram_parameter("pcrep", [32, BL], F16, isOutput=False)
    out_d = nc.declare_dram_parameter("out4", [4, BL], F32, isOutput=True)

    with TileContext(nc) as tc:
        with (
            tc.tile_pool(name="const", bufs=1) as cp,
            tc.tile_pool(name="work", bufs=3) as wp,
            tc.tile_pool(name="grp", bufs=4) as gp,
            tc.tile_pool(name="tail", bufs=2) as tp,
            tc.tile_pool(name="psh", bufs=3, space="PSUM") as psh,
            tc.tile_pool(name="pss", bufs=3, space="PSUM") as pss,
        ):
            # ---- inputs: x first (the count path only needs x), table last
            x_raw = cp.tile([128, T, NSLOT], F16)
            nc.sync.dma_start(out=x_raw, in_=x_d[:])
            sm_raw = cp.tile([32, SM_COLS], F16)
            nc.sync.dma_start(out=sm_raw, in_=sm_d[:])
            pc_raw = cp.tile([32, BL], F16)
            nc.sync.dma_start(out=pc_raw, in_=pc_d[:])
            wc_sb = cp.tile([128, wc_cols], F16)
            nc.sync.dma_start(out=wc_sb, in_=wc_d[:])

            # Vector-engine staging copies absorb DMA waits for the
            # TensorScalarPtr consumers (<=1 wait in their encoding).
            sm_sb = cp.tile([32, SM_COLS], F16)
            nc.vector.tensor_copy(sm_sb, sm_raw)
            pc_sb = cp.tile([32, BL], F16)
            nc.vector.tensor_copy(pc_sb, pc_raw)
            cons32 = cp.tile([32, 4], F32)
            nc.vector.tensor_copy(cons32, sm_sb[:, ds(SM_FC2B, 4)])

            final_sb = cp.tile([4, BL], F32)
            oh_sb = cp.tile([32, BL], F16)

            def build_oh():
                # bucket one-hot, whole batch: OH[p, b] = (bucket_b == p % 8)
                # v = ((pc-2)*8 + 0.5) / 30 ; bucket = clip(floor(v), 0, 7)
                v_sb = cp.tile([32, BL], F16)
                nc.vector.tensor_scalar(v_sb, pc_sb, 8.0 / 30.0, -15.5 / 30.0, OP.mult, OP.add)
                ge_sb = cp.tile([32, BL], F16)
                nc.vector.tensor_scalar(ge_sb, v_sb, cons32[:, ds(2, 1)], None, OP.is_ge)
                lt_sb = cp.tile([32, BL], F16)
                nc.vector.tensor_scalar(lt_sb, v_sb, cons32[:, ds(3, 1)], None, OP.is_lt)
                nc.vector.tensor_tensor(oh_sb, ge_sb, lt_sb, OP.mult)

            # pad mask over the whole batch in one op
            if not with_bias:
                padne_all = cp.tile([128, T, NSLOT], F16)
                nc.vector.tensor_scalar(
                    padne_all, x_raw, float(PAD_IDX), None, OP.not_equal
                )

            def count_path(tiles, toff):
                """Count-path for `tiles` consecutive 128-sample tiles starting
                at tile `toff`: EQ -> T -> idx -> scatter -> xbar transpose.
                Returns the per-tile C tiles' transposes already issued into
                the CT tile given. T_j (total multiplicity of x_j) is
                scattered at column x_j for EVERY slot; equal-valued slots
                carry equal T, so the scatter's last-write-wins duplicate
                handling still ends at column x = T. Pads map to idx -1
                (skipped) in the 6-chunk layout; with bias the pad column's
                W row is 0."""
                xs = x_raw[:, ds(toff, tiles), :]
                EQ = wp.tile([128, tiles, NSLOT, NSLOT], I8, tag="EQ")
                nc.vector.tensor_tensor(
                    EQ,
                    xs[:, :, :, None].broadcast_to([128, tiles, NSLOT, NSLOT]),
                    xs[:, :, None, :].broadcast_to([128, tiles, NSLOT, NSLOT]),
                    OP.is_equal,
                )
                data_f = wp.tile([128, tiles, nidx], F16, tag="data")
                with nc.allow_low_precision(reason="counts <= 32 exact in fp16"):
                    nc.vector.tensor_reduce(
                        data_f[:, :, ds(0, NSLOT)], EQ, mybir.AxisListType.X, OP.add
                    )
                idxs_i = wp.tile([128, tiles, nidx], I16, tag="idxs")
                if not with_bias:
                    idxm = wp.tile([128, tiles, NSLOT], F16, tag="idxm")
                    nc.vector.scalar_tensor_tensor(
                        out=idxm, in0=xs, scalar=1.0,
                        in1=padne_all[:, ds(toff, tiles), :],
                        op0=OP.add, op1=OP.mult,
                    )
                    nc.vector.tensor_scalar(
                        idxs_i[:, :, ds(0, NSLOT)], idxm, -1.0, None, OP.add
                    )
                else:
                    nc.vector.tensor_scalar(
                        idxs_i[:, :, ds(0, NSLOT)], xs, 0.0, None, OP.add
                    )
                    nc.vector.memset(idxs_i[:, :, ds(NSLOT, 1)], vocab - 1)
                    nc.vector.memset(idxs_i[:, :, ds(NSLOT + 1, 1)], -1)
                    nc.vector.memset(data_f[:, :, ds(NSLOT, 2)], 1.0)
                return data_f, idxs_i

            def scatter_transpose(CT, data_f, idxs_i, ti, cti):
                C = wp.tile([128, vocab], F16, tag="C")
                nc.gpsimd.local_scatter(
                    C, data_f[:, ti, :], idxs_i[:, ti, :],
                    channels=128, num_elems=vocab, num_idxs=nidx,
                )
                # xbar DMA transpose: CT[p, c, b] = C[b, c*128 + p]
                nc.sync.dma_start_transpose(
                    out=CT[:, :, ds(128 * cti, 128)], in_=C[:]
                )

            for g, grp in enumerate(group_sizes):
                gw = 128 * grp
                goff = g * grp
                gboff = 128 * goff
                CT = gp.tile([128, nchunk, GW], F16, tag="CT")
                if g == 0:
                    # single-tile count paths: the first C^T slice (and hence
                    # the first matmul) is ready as early as possible
                    for ti in range(grp):
                        d1, i1 = count_path(1, ti)
                        scatter_transpose(CT, d1, i1, 0, ti)
                    build_oh()
                else:
                    for ti in range(grp):
                        d1, i1 = count_path(1, goff + ti)
                        scatter_transpose(CT, d1, i1, 0, ti)

                # ---- h^T = W_pad^T @ C^T, W-stationary, 512-wide moving side
                hq = gp.tile([128, 8, GW], F16, tag="hq")
                for ht in range(8):
                    ph = psh.tile([128, gw], F32, tag="h")
                    for c in range(nchunk):
                        nc.tensor.matmul(
                            ph[:],
                            wc_sb[:, ds(off_w + HIDDEN * c + 128 * ht, 128)],
                            CT[:, c, ds(0, gw)],
                            start=(c == 0),
                            stop=(c == nchunk - 1),
                        )
                    # screlu: relu on Scalar (psum read), min on Vector,
                    # square on Scalar — keeps the psum-evac pass off Vector
                    hr = wp.tile([128, gw], F16, tag="hr")
                    nc.scalar.activation(hr, ph, mybir.ActivationFunctionType.Relu)
                    hs = wp.tile([128, gw], F16, tag="hs")
                    nc.vector.tensor_scalar(hs, hr, 1.0, None, OP.min)
                    nc.scalar.square(out=hq[:, ht, ds(0, gw)], in_=hs)

                # ---- h2 = screlu(fc2T^T @ h^T + fc2_b) as [32, b]
                p2 = pss.tile([32, gw], F32, tag="small")
                for ht in range(8):
                    nc.tensor.matmul(
                        p2[:],
                        wc_sb[:, ds(off_fc2 + H2 * ht, H2)],
                        hq[:, ht, ds(0, gw)],
                        start=(ht == 0),
                        stop=(ht == 7),
                    )
                a2 = tp.tile([32, gw], F16, tag="a2")
                nc.vector.tensor_scalar(a2, p2, cons32[:, ds(0, 1)], 0.0, OP.add, OP.max)
                b2 = tp.tile([32, gw], F16, tag="b2")
                nc.vector.tensor_scalar(b2, a2, 1.0, None, OP.min)
                h2 = tp.tile([32, gw], F16, tag="h2")
                nc.scalar.square(out=h2, in_=b2)

                # ---- heads + bucket-select
                p3 = pss.tile([32, gw], F32, tag="small")
                nc.tensor.matmul(p3[:], sm_sb[:, ds(SM_HW, 32)], h2[:], start=True, stop=True)
                o4 = tp.tile([32, gw], F16, tag="o4")
                nc.vector.tensor_scalar(o4, p3, cons32[:, ds(1, 1)], None, OP.add)
                mk = tp.tile([32, gw], F16, tag="mk")
                nc.vector.tensor_tensor(mk, o4, oh_sb[:, ds(gboff, gw)], OP.mult)
                p4 = pss.tile([4, gw], F32, tag="small")
                nc.tensor.matmul(p4[:], sm_sb[:, ds(SM_SEL, 4)], mk[:], start=True, stop=True)
                nc.vector.tensor_copy(final_sb[:, ds(gboff, gw)], p4[:])
                nc.sync.dma_start(
                    out=out_d[:, ds(gboff, gw)], in_=final_sb[:, ds(gboff, gw)]
                )

    nc.finalize()
    return nc


# ---------------------------------------------------------------------------
# host side


def _prep_shared(emb_w, bias1, fc2_w, fc2_b, cp_w, cp_b, wdl_w, wdl_b, nchunk):
    vocab = nchunk * 128
    emb = np.asarray(emb_w, np.float32).copy()
    emb[PAD_IDX] = 0.0
    wpad = np.zeros((vocab, HIDDEN), np.float32)
    wpad[: min(INPUT_DIM + 1, vocab)] = emb[: min(INPUT_DIM + 1, vocab)]
    if nchunk == 7:
        wpad[vocab - 1] = np.asarray(bias1, np.float32)
    wpad = wpad.reshape(nchunk, 128, HIDDEN).transpose(1, 0, 2)

    fc2t = np.asarray(fc2_w, np.float32).T  # [1024, 32]
    fc2t = fc2t.reshape(8, 128, H2).transpose(1, 0, 2)  # [128, 8, 32]

    off_w = 0
    off_fc2 = nchunk * HIDDEN
    wc_cols = off_fc2 + 8 * H2

    wconst = np.zeros((128, wc_cols), np.float16)
    wconst[:, off_w : off_w + nchunk * HIDDEN] = wpad.reshape(128, -1).astype(np.float16)
    wconst[:, off_fc2 : off_fc2 + 8 * H2] = fc2t.reshape(128, -1).astype(np.float16)

    stacked = np.zeros((32, H2), np.float32)
    stacked_b = np.zeros((32,), np.float32)
    stacked[0:8] = np.asarray(cp_w, np.float32)
    stacked_b[0:8] = np.asarray(cp_b, np.float32)
    for k in range(3):
        for u in range(8):
            stacked[8 + 8 * k + u] = np.asarray(wdl_w, np.float32)[3 * u + k]
            stacked_b[8 + 8 * k + u] = np.asarray(wdl_b, np.float32)[3 * u + k]

    smconst = np.zeros((32, SM_COLS), np.float16)
    smconst[:, SM_HW : SM_HW + 32] = stacked.T.astype(np.float16)  # [H2, 32]
    smconst[:, SM_FC2B] = np.asarray(fc2_b, np.float32).astype(np.float16)
    smconst[:, SM_HEADB] = stacked_b.astype(np.float16)
    uu = np.arange(32) % 8
    smconst[:, SM_LO] = np.where(uu == 0, -30000.0, uu).astype(np.float16)
    smconst[:, SM_HI] = np.where(uu == 7, 30000.0, uu + 1).astype(np.float16)
    sel = np.zeros((32, 4), np.float16)
    sel[np.arange(32), np.arange(32) // 8] = 1.0
    smconst[:, SM_SEL : SM_SEL + 4] = sel

    return dict(wconst=wconst, smconst=smconst)


def _prep_core(x_c, pc_c):
    BL = x_c.shape[0]
    T = BL // 128
    x16 = np.ascontiguousarray(
        np.asarray(x_c, np.int64)
        .astype(np.float16)
        .reshape(T, 128, NSLOT)
        .transpose(1, 0, 2)
    )
    pcrep = np.broadcast_to(
        np.asarray(pc_c, np.int64).astype(np.float16)[None, :], (32, BL)
    ).copy()
    return dict(x16=x16, pcrep=pcrep)


_NC_CACHE = {}


def kernel(x, piece_count, emb_w, bias1, fc2_w, fc2_b, cp_w, cp_b, wdl_w, wdl_b):
    x = np.asarray(x)
    piece_count = np.asarray(piece_count)
    B = x.shape[0]
    BL = B // NCORES
    nchunk = 6 if not np.any(np.asarray(bias1)) else 7

    key = (BL, nchunk)
    if key not in _NC_CACHE:
        _NC_CACHE[key] = build_nc(BL, nchunk)
    nc = _NC_CACHE[key]

    shared = _prep_shared(emb_w, bias1, fc2_w, fc2_b, cp_w, cp_b, wdl_w, wdl_b, nchunk)
    in_maps = []
    for c in range(NCORES):
        m = dict(shared)
        m.update(_prep_core(x[c * BL : (c + 1) * BL], piece_count[c * BL : (c + 1) * BL]))
        in_maps.append(m)

    res = bass_utils.run_bass_kernel_spmd(nc, in_maps, list(range(NCORES))).results
    out4 = np.concatenate([res[c]["out4"] for c in range(NCORES)], axis=1)  # [4, B]
    outT = out4.T.astype(np.float32)
    cp_out = np.ascontiguousarray(outT[:, 0:1])
    wdl_out = np.ascontiguousarray(outT[:, 1:4])
    return cp_out, wdl_out


# revision 32
# speedup vs baseline: 1.1350x; 1.0276x over previous
"""Trainium2 Bass kernel for EvalNet (EmbeddingBag + MLP + bucketed heads).

Strategy (pure data parallel over 8 cores, batch dim sharded):
  The EmbeddingBag-sum  h[b] = sum_j emb_w[x[b,j]]  is reformulated as a
  dense matmul  h^T = W_pad^T @ C^T  where C[b, v] counts how many of the
  32 slots of sample b hold index v.  Because emb_w's padding row (768)
  is zero and rows 769+ don't exist, the effective vocab is 768 = 6*128
  when bias1 == 0 (the usual case); with a nonzero bias1 a 7th chunk
  carries bias1 in row 895 and a constant-1 column is injected.

  Per 128-sample tile:
    GpSimd: EQ[p,j,j'] = (x_j == x_j') (int8), strict-upper mask via
            affine_select, and the final per-partition local_scatter that
            writes the multiplicity T_j of each slot's value at column
            x_j for the LAST occurrence only (idx -1 elsewhere, skipped).
    Vector: the two add-reduces (T, ngt), the last-occurrence/index
            arithmetic, and the psum evacuations.
    PE:     chunkwise 128x128 transposes of C into a 4-tile group buffer
            C^T [128, nchunk, 512], then the W-stationary table matmul
            producing h^T [hid, b] directly (no second transpose), fc2,
            stacked cp/wdl heads, and the bucket block-sum selection.
    Scalar: the screlu squares (exact: PWP tables represent x^2).

  Only 5 DMA instructions are issued (2 packed const loads, x, pc, out);
  multi-sem waits are split by Bacc's generate_event_semaphores pass.
"""

import sys

sys.path.insert(0, "/opt/trn_rl_repo")

import numpy as np

import concourse.bacc as bacc
import concourse.mybir as mybir
from concourse import bass_utils
from concourse.bass import ds
from concourse.tile import TileContext

F16 = mybir.dt.float16
F32 = mybir.dt.float32
I16 = mybir.dt.int16
I8 = mybir.dt.int8
OP = mybir.AluOpType

INPUT_DIM = 768
HIDDEN = 1024
H2 = 32
PAD_IDX = 768
NCORES = 8
NSLOT = 32
GRP = 4  # 128-sample tiles per matmul group (512-wide moving side)

# small const layout (fp16, 32 partitions)
SM_HW = 0                  # [32, 32] head weightsT
SM_FC2B = 32
SM_HEADB = 33
SM_LO = 34
SM_HI = 35
SM_SEL = 36                # [32, 4]
SM_COLS = 40


def build_nc(BL, nchunk):
    """Build the Bass module for one core processing BL samples.

    nchunk: 6 when bias1 == 0 (vocab 768, pad index skipped), else 7
    (vocab 896 with bias1 in row 895 via a constant-1 column).
    """
    T = BL // 128
    g0 = min(GRP, T)
    group_sizes = [g0] * (T // g0)
    assert T % g0 == 0
    GW = 128 * max(group_sizes)
    vocab = nchunk * 128
    with_bias = nchunk == 7
    nidx = NSLOT + 2 if with_bias else NSLOT

    # packed const layout (fp16, 128 partitions)
    off_w = 0                      # [128, nchunk*1024] table chunks
    off_fc2 = nchunk * HIDDEN      # [128, 8*32] fc2^T chunks
    wc_cols = off_fc2 + 8 * H2

    # Bacc (not plain Bass): its finalize() runs generate_event_semaphores,
    # which splits multi-sem waits down to the 1-wait-per-instruction TPB
    # encoding limit — walrus codegen rejects unsplit Tile output.
    nc = bacc.Bacc("TRN2", target_bir_lowering=False)

    wc_d = nc.declare_dram_parameter("wconst", [128, wc_cols], F16, isOutput=False)
    sm_d = nc.declare_dram_parameter("smconst", [32, SM_COLS], F16, isOutput=False)
    x_d = nc.declare_dram_parameter("x16", [128, T, NSLOT], F16, isOutput=False)
    pc_d = nc.declare_dram_parameter("pcrep", [32, BL], F16, isOutput=False)
    out_d = nc.declare_dram_parameter("out4", [4, BL], F32, isOutput=True)

    with TileContext(nc) as tc:
        with (
            tc.tile_pool(name="const", bufs=1) as cp,
            tc.tile_pool(name="work", bufs=3) as wp,
            tc.tile_pool(name="grp", bufs=4) as gp,
            tc.tile_pool(name="tail", bufs=2) as tp,
            tc.tile_pool(name="psh", bufs=3, space="PSUM") as psh,
            tc.tile_pool(name="pss", bufs=4, space="PSUM") as pss,
        ):
            # ---- inputs: x first (the count path only needs x), table last
            x_raw = cp.tile([128, T, NSLOT], F16)
            nc.sync.dma_start(out=x_raw, in_=x_d[:])
            sm_raw = cp.tile([32, SM_COLS], F16)
            nc.sync.dma_start(out=sm_raw, in_=sm_d[:])
            pc_raw = cp.tile([32, BL], F16)
            nc.sync.dma_start(out=pc_raw, in_=pc_d[:])
            wc_sb = cp.tile([128, wc_cols], F16)
            nc.sync.dma_start(out=wc_sb, in_=wc_d[:])

            # Vector-engine staging copies absorb DMA waits for the
            # TensorScalarPtr consumers (<=1 wait in their encoding).
            sm_sb = cp.tile([32, SM_COLS], F16)
            nc.vector.tensor_copy(sm_sb, sm_raw)
            pc_sb = cp.tile([32, BL], F16)
            nc.vector.tensor_copy(pc_sb, pc_raw)
            cons32 = cp.tile([32, 4], F32)
            nc.vector.tensor_copy(cons32, sm_sb[:, ds(SM_FC2B, 4)])

            final_sb = cp.tile([4, BL], F32)
            oh_sb = cp.tile([32, BL], F16)

            def build_oh():
                # bucket one-hot, whole batch: OH[p, b] = (bucket_b == p % 8)
                # v = ((pc-2)*8 + 0.5) / 30 ; bucket = clip(floor(v), 0, 7)
                v_sb = cp.tile([32, BL], F16)
                nc.vector.tensor_scalar(v_sb, pc_sb, 8.0 / 30.0, -15.5 / 30.0, OP.mult, OP.add)
                ge_sb = cp.tile([32, BL], F16)
                nc.vector.tensor_scalar(ge_sb, v_sb, cons32[:, ds(2, 1)], None, OP.is_ge)
                lt_sb = cp.tile([32, BL], F16)
                nc.vector.tensor_scalar(lt_sb, v_sb, cons32[:, ds(3, 1)], None, OP.is_lt)
                nc.vector.tensor_tensor(oh_sb, ge_sb, lt_sb, OP.mult)

            # pad mask over the whole batch in one op
            if not with_bias:
                padne_all = cp.tile([128, T, NSLOT], F16)
                nc.vector.tensor_scalar(
                    padne_all, x_raw, float(PAD_IDX), None, OP.not_equal
                )

            def count_path(tiles, toff):
                """Count-path for `tiles` consecutive 128-sample tiles starting
                at tile `toff`: EQ -> T -> idx -> scatter -> xbar transpose.
                Returns the per-tile C tiles' transposes already issued into
                the CT tile given. T_j (total multiplicity of x_j) is
                scattered at column x_j for EVERY slot; equal-valued slots
                carry equal T, so the scatter's last-write-wins duplicate
                handling still ends at column x = T. Pads map to idx -1
                (skipped) in the 6-chunk layout; with bias the pad column's
                W row is 0."""
                xs = x_raw[:, ds(toff, tiles), :]
                EQ = wp.tile([128, tiles, NSLOT, NSLOT], I8, tag="EQ")
                nc.vector.tensor_tensor(
                    EQ,
                    xs[:, :, :, None].broadcast_to([128, tiles, NSLOT, NSLOT]),
                    xs[:, :, None, :].broadcast_to([128, tiles, NSLOT, NSLOT]),
                    OP.is_equal,
                )
                data_f = wp.tile([128, tiles, nidx], F16, tag="data")
                with nc.allow_low_precision(reason="counts <= 32 exact in fp16"):
                    nc.vector.tensor_reduce(
                        data_f[:, :, ds(0, NSLOT)], EQ, mybir.AxisListType.X, OP.add
                    )
                idxs_i = wp.tile([128, tiles, nidx], I16, tag="idxs")
                if not with_bias:
                    idxm = wp.tile([128, tiles, NSLOT], F16, tag="idxm")
                    nc.vector.scalar_tensor_tensor(
                        out=idxm, in0=xs, scalar=1.0,
                        in1=padne_all[:, ds(toff, tiles), :],
                        op0=OP.add, op1=OP.mult,
                    )
                    nc.vector.tensor_scalar(
                        idxs_i[:, :, ds(0, NSLOT)], idxm, -1.0, None, OP.add
                    )
                else:
                    nc.vector.tensor_scalar(
                        idxs_i[:, :, ds(0, NSLOT)], xs, 0.0, None, OP.add
                    )
                    nc.vector.memset(idxs_i[:, :, ds(NSLOT, 1)], vocab - 1)
                    nc.vector.memset(idxs_i[:, :, ds(NSLOT + 1, 1)], -1)
                    nc.vector.memset(data_f[:, :, ds(NSLOT, 2)], 1.0)
                return data_f, idxs_i

            def scatter_transpose(CT, data_f, idxs_i, ti, cti):
                C = wp.tile([128, vocab], F16, tag="C")
                nc.gpsimd.local_scatter(
                    C, data_f[:, ti, :], idxs_i[:, ti, :],
                    channels=128, num_elems=vocab, num_idxs=nidx,
                )
                # xbar DMA transpose: CT[p, c, b] = C[b, c*128 + p]
                nc.sync.dma_start_transpose(
                    out=CT[:, :, ds(128 * cti, 128)], in_=C[:]
                )

            for g, grp in enumerate(group_sizes):
                gw = 128 * grp
                goff = g * grp
                gboff = 128 * goff
                CT = gp.tile([128, nchunk, GW], F16, tag="CT")
                if g == 0:
                    # single-tile count paths: the first C^T slice (and hence
                    # the first matmul) is ready as early as possible
                    for ti in range(grp):
                        d1, i1 = count_path(1, ti)
                        scatter_transpose(CT, d1, i1, 0, ti)
                    build_oh()
                else:
                    for ti in range(grp):
                        d1, i1 = count_path(1, goff + ti)
                        scatter_transpose(CT, d1, i1, 0, ti)

                # ---- h^T = W_pad^T @ C^T, W-stationary, 512-wide moving side
                hq = gp.tile([128, 8, GW], F16, tag="hq")
                for ht in range(8):
                    ph = psh.tile([128, gw], F32, tag="h")
                    for c in range(nchunk):
                        nc.tensor.matmul(
                            ph[:],
                            wc_sb[:, ds(off_w + HIDDEN * c + 128 * ht, 128)],
                            CT[:, c, ds(0, gw)],
                            start=(c == 0),
                            stop=(c == nchunk - 1),
                        )
                    # screlu: relu on Scalar (psum read), then either
                    # min(Vector) + square(Scalar), or a pure-Scalar chain
                    # clip(x,0,1)^2 = (1 - relu(1 - relu(x)))^2 — alternate
                    # per ht to balance the two engines.
                    hr = wp.tile([128, gw], F16, tag="hr")
                    nc.scalar.activation(hr, ph, mybir.ActivationFunctionType.Relu)
                    if ht % 2 == 0:
                        hs = wp.tile([128, gw], F16, tag="hs")
                        nc.vector.tensor_scalar(hs, hr, 1.0, None, OP.min)
                        nc.scalar.square(out=hq[:, ht, ds(0, gw)], in_=hs)
                    else:
                        hs = wp.tile([128, gw], F16, tag="hs")
                        nc.scalar.activation(
                            hs, hr, mybir.ActivationFunctionType.Relu,
                            bias=1.0, scale=-1.0,
                        )
                        nc.scalar.activation(
                            hq[:, ht, ds(0, gw)], hs,
                            mybir.ActivationFunctionType.Square,
                            bias=1.0, scale=-1.0,
                        )

                # ---- h2 = screlu(fc2T^T @ h^T + fc2_b) as [32, b]
                p2 = pss.tile([32, gw], F32, tag="small")
                for ht in range(8):
                    nc.tensor.matmul(
                        p2[:],
                        wc_sb[:, ds(off_fc2 + H2 * ht, H2)],
                        hq[:, ht, ds(0, gw)],
                        start=(ht == 0),
                        stop=(ht == 7),
                    )
                a2 = tp.tile([32, gw], F16, tag="a2")
                nc.vector.tensor_scalar(a2, p2, cons32[:, ds(0, 1)], 0.0, OP.add, OP.max)
                b2 = tp.tile([32, gw], F16, tag="b2")
                nc.vector.tensor_scalar(b2, a2, 1.0, None, OP.min)
                h2 = tp.tile([32, gw], F16, tag="h2")
                nc.scalar.square(out=h2, in_=b2)

                # ---- heads + bucket-select
                p3 = pss.tile([32, gw], F32, tag="small")
                nc.tensor.matmul(p3[:], sm_sb[:, ds(SM_HW, 32)], h2[:], start=True, stop=True)
                o4 = tp.tile([32, gw], F16, tag="o4")
                nc.vector.tensor_scalar(o4, p3, cons32[:, ds(1, 1)], None, OP.add)
                mk = tp.tile([32, gw], F16, tag="mk")
                nc.vector.tensor_tensor(mk, o4, oh_sb[:, ds(gboff, gw)], OP.mult)
                p4 = pss.tile([4, gw], F32, tag="small")
                nc.tensor.matmul(p4[:], sm_sb[:, ds(SM_SEL, 4)], mk[:], start=True, stop=True)
                nc.vector.tensor_copy(final_sb[:, ds(gboff, gw)], p4[:])
                nc.sync.dma_start(
                    out=out_d[:, ds(gboff, gw)], in_=final_sb[:, ds(gboff, gw)]
                )

    nc.finalize()
    return nc


# ---------------------------------------------------------------------------
# host side


def _prep_shared(emb_w, bias1, fc2_w, fc2_b, cp_w, cp_b, wdl_w, wdl_b, nchunk):
    vocab = nchunk * 128
    emb = np.asarray(emb_w, np.float32).copy()
    emb[PAD_IDX] = 0.0
    wpad = np.zeros((vocab, HIDDEN), np.float32)
    wpad[: min(INPUT_DIM + 1, vocab)] = emb[: min(INPUT_DIM + 1, vocab)]
    if nchunk == 7:
        wpad[vocab - 1] = np.asarray(bias1, np.float32)
    wpad = wpad.reshape(nchunk, 128, HIDDEN).transpose(1, 0, 2)

    fc2t = np.asarray(fc2_w, np.float32).T  # [1024, 32]
    fc2t = fc2t.reshape(8, 128, H2).transpose(1, 0, 2)  # [128, 8, 32]

    off_w = 0
    off_fc2 = nchunk * HIDDEN
    wc_cols = off_fc2 + 8 * H2

    wconst = np.zeros((128, wc_cols), np.float16)
    wconst[:, off_w : off_w + nchunk * HIDDEN] = wpad.reshape(128, -1).astype(np.float16)
    wconst[:, off_fc2 : off_fc2 + 8 * H2] = fc2t.reshape(128, -1).astype(np.float16)

    stacked = np.zeros((32, H2), np.float32)
    stacked_b = np.zeros((32,), np.float32)
    stacked[0:8] = np.asarray(cp_w, np.float32)
    stacked_b[0:8] = np.asarray(cp_b, np.float32)
    for k in range(3):
        for u in range(8):
            stacked[8 + 8 * k + u] = np.asarray(wdl_w, np.float32)[3 * u + k]
            stacked_b[8 + 8 * k + u] = np.asarray(wdl_b, np.float32)[3 * u + k]

    smconst = np.zeros((32, SM_COLS), np.float16)
    smconst[:, SM_HW : SM_HW + 32] = stacked.T.astype(np.float16)  # [H2, 32]
    smconst[:, SM_FC2B] = np.asarray(fc2_b, np.float32).astype(np.float16)
    smconst[:, SM_HEADB] = stacked_b.astype(np.float16)
    uu = np.arange(32) % 8
    smconst[:, SM_LO] = np.where(uu == 0, -30000.0, uu).astype(np.float16)
    smconst[:, SM_HI] = np.where(uu == 7, 30000.0, uu + 1).astype(np.float16)
    sel = np.zeros((32, 4), np.float16)
    sel[np.arange(32), np.arange(32) // 8] = 1.0
    smconst[:, SM_SEL : SM_SEL + 4] = sel

    return dict(wconst=wconst, smconst=smconst)


def _prep_core(x_c, pc_c):
    BL = x_c.shape[0]
    T = BL // 128
    x16 = np.ascontiguousarray(
        np.asarray(x_c, np.int64)
        .astype(np.float16)
        .reshape(T, 128, NSLOT)
        .transpose(1, 0, 2)
    )
    pcrep = np.broadcast_to(
        np.asarray(pc_c, np.int64).astype(np.float16)[None, :], (32, BL)
    ).copy()
    return dict(x16=x16, pcrep=pcrep)


_NC_CACHE = {}


def kernel(x, piece_count, emb_w, bias1, fc2_w, fc2_b, cp_w, cp_b, wdl_w, wdl_b):
    x = np.asarray(x)
    piece_count = np.asarray(piece_count)
    B = x.shape[0]
    BL = B // NCORES
    nchunk = 6 if not np.any(np.asarray(bias1)) else 7

    key = (BL, nchunk)
    if key not in _NC_CACHE:
        _NC_CACHE[key] = build_nc(BL, nchunk)
    nc = _NC_CACHE[key]

    shared = _prep_shared(emb_w, bias1, fc2_w, fc2_b, cp_w, cp_b, wdl_w, wdl_b, nchunk)
    in_maps = []
    for c in range(NCORES):
        m = dict(shared)
        m.update(_prep_core(x[c * BL : (c + 1) * BL], piece_count[c * BL : (c + 1) * BL]))
        in_maps.append(m)

    res = bass_utils.run_bass_kernel_spmd(nc, in_maps, list(range(NCORES))).results
    out4 = np.concatenate([res[c]["out4"] for c in range(NCORES)], axis=1)  # [4, B]
    outT = out4.T.astype(np.float32)
    cp_out = np.ascontiguousarray(outT[:, 0:1])
    wdl_out = np.ascontiguousarray(outT[:, 1:4])
    return cp_out, wdl_out


# revision 34
# speedup vs baseline: 1.1515x; 1.0145x over previous
"""Trainium2 Bass kernel for EvalNet (EmbeddingBag + MLP + bucketed heads).

Strategy (pure data parallel over 8 cores, batch dim sharded):
  The EmbeddingBag-sum  h[b] = sum_j emb_w[x[b,j]]  is reformulated as a
  dense matmul  h^T = W_pad^T @ C^T  where C[b, v] counts how many of the
  32 slots of sample b hold index v.  Because emb_w's padding row (768)
  is zero and rows 769+ don't exist, the effective vocab is 768 = 6*128
  when bias1 == 0 (the usual case); with a nonzero bias1 a 7th chunk
  carries bias1 in row 895 and a constant-1 column is injected.

  Per 128-sample tile:
    GpSimd: EQ[p,j,j'] = (x_j == x_j') (int8), strict-upper mask via
            affine_select, and the final per-partition local_scatter that
            writes the multiplicity T_j of each slot's value at column
            x_j for the LAST occurrence only (idx -1 elsewhere, skipped).
    Vector: the two add-reduces (T, ngt), the last-occurrence/index
            arithmetic, and the psum evacuations.
    PE:     chunkwise 128x128 transposes of C into a 4-tile group buffer
            C^T [128, nchunk, 512], then the W-stationary table matmul
            producing h^T [hid, b] directly (no second transpose), fc2,
            stacked cp/wdl heads, and the bucket block-sum selection.
    Scalar: the screlu squares (exact: PWP tables represent x^2).

  Only 5 DMA instructions are issued (2 packed const loads, x, pc, out);
  multi-sem waits are split by Bacc's generate_event_semaphores pass.
"""

import sys

sys.path.insert(0, "/opt/trn_rl_repo")

import numpy as np

import concourse.bacc as bacc
import concourse.mybir as mybir
from concourse import bass_utils
from concourse.bass import ds
from concourse.tile import TileContext

F16 = mybir.dt.float16
F32 = mybir.dt.float32
I16 = mybir.dt.int16
I8 = mybir.dt.int8
OP = mybir.AluOpType

INPUT_DIM = 768
HIDDEN = 1024
H2 = 32
PAD_IDX = 768
NCORES = 8
NSLOT = 32
GRP = 4  # 128-sample tiles per matmul group (512-wide moving side)

# small const layout (fp16, 32 partitions)
SM_HW = 0                  # [32, 32] head weightsT
SM_FC2B = 32
SM_HEADB = 33
SM_LO = 34
SM_HI = 35
SM_SEL = 36                # [32, 4]
SM_COLS = 40


def build_nc(BL, nchunk):
    """Build the Bass module for one core processing BL samples.

    nchunk: 6 when bias1 == 0 (vocab 768, pad index skipped), else 7
    (vocab 896 with bias1 in row 895 via a constant-1 column).
    """
    T = BL // 128
    g0 = min(GRP, T)
    group_sizes = [g0] * (T // g0)
    assert T % g0 == 0
    GW = 128 * max(group_sizes)
    vocab = nchunk * 128
    with_bias = nchunk == 7
    nidx = NSLOT + 2 if with_bias else NSLOT

    # packed const layout (fp16, 128 partitions)
    off_w = 0                      # [128, nchunk*1024] table chunks
    off_fc2 = nchunk * HIDDEN      # [128, 8*32] fc2^T chunks
    wc_cols = off_fc2 + 8 * H2

    # Bacc (not plain Bass): its finalize() runs generate_event_semaphores,
    # which splits multi-sem waits down to the 1-wait-per-instruction TPB
    # encoding limit — walrus codegen rejects unsplit Tile output.
    nc = bacc.Bacc("TRN2", target_bir_lowering=False)

    wc_d = nc.declare_dram_parameter("wconst", [128, wc_cols], F16, isOutput=False)
    sm_d = nc.declare_dram_parameter("smconst", [32, SM_COLS], F16, isOutput=False)
    x_d = nc.declare_dram_parameter("x16", [128, T, NSLOT], F16, isOutput=False)
    pc_d = nc.declare_dram_parameter("pcrep", [32, BL], F16, isOutput=False)
    out_d = nc.declare_dram_parameter("out4", [4, BL], F32, isOutput=True)

    with TileContext(nc) as tc:
        with (
            tc.tile_pool(name="const", bufs=1) as cp,
            tc.tile_pool(name="work", bufs=3) as wp,
            tc.tile_pool(name="grp", bufs=4) as gp,
            tc.tile_pool(name="tail", bufs=2) as tp,
            tc.tile_pool(name="psh", bufs=3, space="PSUM") as psh,
            tc.tile_pool(name="pss", bufs=4, space="PSUM") as pss,
        ):
            # ---- inputs: x first (the count path only needs x), table last
            x_raw = cp.tile([128, T, NSLOT], F16)
            nc.sync.dma_start(out=x_raw, in_=x_d[:])
            sm_raw = cp.tile([32, SM_COLS], F16)
            nc.sync.dma_start(out=sm_raw, in_=sm_d[:])
            pc_raw = cp.tile([32, BL], F16)
            nc.sync.dma_start(out=pc_raw, in_=pc_d[:])
            wc_sb = cp.tile([128, wc_cols], F16)
            nc.sync.dma_start(out=wc_sb, in_=wc_d[:])

            # Vector-engine staging copies absorb DMA waits for the
            # TensorScalarPtr consumers (<=1 wait in their encoding).
            sm_sb = cp.tile([32, SM_COLS], F16)
            nc.vector.tensor_copy(sm_sb, sm_raw)
            pc_sb = cp.tile([32, BL], F16)
            nc.vector.tensor_copy(pc_sb, pc_raw)
            cons32 = cp.tile([32, 4], F32)
            nc.vector.tensor_copy(cons32, sm_sb[:, ds(SM_FC2B, 4)])

            final_sb = cp.tile([4, BL], F32)
            oh_sb = cp.tile([32, BL], F16)

            def build_oh():
                # bucket one-hot, whole batch: OH[p, b] = (bucket_b == p % 8)
                # v = ((pc-2)*8 + 0.5) / 30 ; bucket = clip(floor(v), 0, 7)
                v_sb = cp.tile([32, BL], F16)
                nc.vector.tensor_scalar(v_sb, pc_sb, 8.0 / 30.0, -15.5 / 30.0, OP.mult, OP.add)
                ge_sb = cp.tile([32, BL], F16)
                nc.vector.tensor_scalar(ge_sb, v_sb, cons32[:, ds(2, 1)], None, OP.is_ge)
                lt_sb = cp.tile([32, BL], F16)
                nc.vector.tensor_scalar(lt_sb, v_sb, cons32[:, ds(3, 1)], None, OP.is_lt)
                nc.vector.tensor_tensor(oh_sb, ge_sb, lt_sb, OP.mult)

            # pad mask over the whole batch in one op
            if not with_bias:
                padne_all = cp.tile([128, T, NSLOT], F16)
                nc.vector.tensor_scalar(
                    padne_all, x_raw, float(PAD_IDX), None, OP.not_equal
                )

            def count_path(tiles, toff):
                """Count-path for `tiles` consecutive 128-sample tiles starting
                at tile `toff`: EQ -> T -> idx -> scatter -> xbar transpose.
                Returns the per-tile C tiles' transposes already issued into
                the CT tile given. T_j (total multiplicity of x_j) is
                scattered at column x_j for EVERY slot; equal-valued slots
                carry equal T, so the scatter's last-write-wins duplicate
                handling still ends at column x = T. Pads map to idx -1
                (skipped) in the 6-chunk layout; with bias the pad column's
                W row is 0."""
                xs = x_raw[:, ds(toff, tiles), :]
                EQ = wp.tile([128, tiles, NSLOT, NSLOT], I8, tag="EQ")
                nc.vector.tensor_tensor(
                    EQ,
                    xs[:, :, :, None].broadcast_to([128, tiles, NSLOT, NSLOT]),
                    xs[:, :, None, :].broadcast_to([128, tiles, NSLOT, NSLOT]),
                    OP.is_equal,
                )
                data_f = wp.tile([128, tiles, nidx], F16, tag="data")
                with nc.allow_low_precision(reason="counts <= 32 exact in fp16"):
                    nc.vector.tensor_reduce(
                        data_f[:, :, ds(0, NSLOT)], EQ, mybir.AxisListType.X, OP.add
                    )
                idxs_i = wp.tile([128, tiles, nidx], I16, tag="idxs")
                if not with_bias:
                    idxm = wp.tile([128, tiles, NSLOT], F16, tag="idxm")
                    nc.vector.scalar_tensor_tensor(
                        out=idxm, in0=xs, scalar=1.0,
                        in1=padne_all[:, ds(toff, tiles), :],
                        op0=OP.add, op1=OP.mult,
                    )
                    nc.vector.tensor_scalar(
                        idxs_i[:, :, ds(0, NSLOT)], idxm, -1.0, None, OP.add
                    )
                else:
                    nc.vector.tensor_scalar(
                        idxs_i[:, :, ds(0, NSLOT)], xs, 0.0, None, OP.add
                    )
                    nc.vector.memset(idxs_i[:, :, ds(NSLOT, 1)], vocab - 1)
                    nc.vector.memset(idxs_i[:, :, ds(NSLOT + 1, 1)], -1)
                    nc.vector.memset(data_f[:, :, ds(NSLOT, 2)], 1.0)
                return data_f, idxs_i

            def scatter_transpose(CT, data_f, idxs_i, ti, cti):
                C = wp.tile([128, vocab], F16, tag="C")
                nc.gpsimd.local_scatter(
                    C, data_f[:, ti, :], idxs_i[:, ti, :],
                    channels=128, num_elems=vocab, num_idxs=nidx,
                )
                # xbar DMA transpose: CT[p, c, b] = C[b, c*128 + p]
                nc.sync.dma_start_transpose(
                    out=CT[:, :, ds(128 * cti, 128)], in_=C[:]
                )

            for g, grp in enumerate(group_sizes):
                gw = 128 * grp
                goff = g * grp
                gboff = 128 * goff
                CT = gp.tile([128, nchunk, GW], F16, tag="CT")
                if g == 0:
                    # single-tile count paths: the first C^T slice (and hence
                    # the first matmul) is ready as early as possible
                    for ti in range(grp):
                        d1, i1 = count_path(1, ti)
                        scatter_transpose(CT, d1, i1, 0, ti)
                    build_oh()
                else:
                    for ti in range(grp):
                        d1, i1 = count_path(1, goff + ti)
                        scatter_transpose(CT, d1, i1, 0, ti)

                # ---- h^T = W_pad^T @ C^T, W-stationary, 512-wide moving side
                # First group runs in two half-width passes: the extra
                # ldweights land in the ramp window while the PE would
                # otherwise idle waiting for the full C^T.
                hq = gp.tile([128, 8, GW], F16, tag="hq")
                halves = [(0, gw // 2), (gw // 2, gw - gw // 2)] if g == 0 else [(0, gw)]
                for hoff, hw_ in halves:
                  for ht in range(8):
                    ph = psh.tile([128, hw_], F32, tag="h")
                    for c in range(nchunk):
                        nc.tensor.matmul(
                            ph[:],
                            wc_sb[:, ds(off_w + HIDDEN * c + 128 * ht, 128)],
                            CT[:, c, ds(hoff, hw_)],
                            start=(c == 0),
                            stop=(c == nchunk - 1),
                        )
                    # screlu: relu on Scalar (psum read), then either
                    # min(Vector) + square(Scalar), or a pure-Scalar chain
                    # clip(x,0,1)^2 = (1 - relu(1 - relu(x)))^2 — alternate
                    # per ht to balance the two engines.
                    hr = wp.tile([128, hw_], F16, tag="hr")
                    nc.scalar.activation(hr, ph, mybir.ActivationFunctionType.Relu)
                    if ht % 2 == 0:
                        hs = wp.tile([128, hw_], F16, tag="hs")
                        nc.vector.tensor_scalar(hs, hr, 1.0, None, OP.min)
                        nc.scalar.square(out=hq[:, ht, ds(hoff, hw_)], in_=hs)
                    else:
                        hs = wp.tile([128, hw_], F16, tag="hs")
                        nc.scalar.activation(
                            hs, hr, mybir.ActivationFunctionType.Relu,
                            bias=1.0, scale=-1.0,
                        )
                        nc.scalar.activation(
                            hq[:, ht, ds(hoff, hw_)], hs,
                            mybir.ActivationFunctionType.Square,
                            bias=1.0, scale=-1.0,
                        )

                # ---- h2 = screlu(fc2T^T @ h^T + fc2_b) as [32, b]
                p2 = pss.tile([32, gw], F32, tag="small")
                for ht in range(8):
                    nc.tensor.matmul(
                        p2[:],
                        wc_sb[:, ds(off_fc2 + H2 * ht, H2)],
                        hq[:, ht, ds(0, gw)],
                        start=(ht == 0),
                        stop=(ht == 7),
                    )
                a2 = tp.tile([32, gw], F16, tag="a2")
                nc.vector.tensor_scalar(a2, p2, cons32[:, ds(0, 1)], 0.0, OP.add, OP.max)
                b2 = tp.tile([32, gw], F16, tag="b2")
                nc.vector.tensor_scalar(b2, a2, 1.0, None, OP.min)
                h2 = tp.tile([32, gw], F16, tag="h2")
                nc.scalar.square(out=h2, in_=b2)

                # ---- heads + bucket-select
                p3 = pss.tile([32, gw], F32, tag="small")
                nc.tensor.matmul(p3[:], sm_sb[:, ds(SM_HW, 32)], h2[:], start=True, stop=True)
                o4 = tp.tile([32, gw], F16, tag="o4")
                nc.vector.tensor_scalar(o4, p3, cons32[:, ds(1, 1)], None, OP.add)
                mk = tp.tile([32, gw], F16, tag="mk")
                nc.vector.tensor_tensor(mk, o4, oh_sb[:, ds(gboff, gw)], OP.mult)
                p4 = pss.tile([4, gw], F32, tag="small")
                nc.tensor.matmul(p4[:], sm_sb[:, ds(SM_SEL, 4)], mk[:], start=True, stop=True)
                nc.vector.tensor_copy(final_sb[:, ds(gboff, gw)], p4[:])
                nc.sync.dma_start(
                    out=out_d[:, ds(gboff, gw)], in_=final_sb[:, ds(gboff, gw)]
                )

    nc.finalize()
    return nc


# ---------------------------------------------------------------------------
# host side


def _prep_shared(emb_w, bias1, fc2_w, fc2_b, cp_w, cp_b, wdl_w, wdl_b, nchunk):
    vocab = nchunk * 128
    emb = np.asarray(emb_w, np.float32).copy()
    emb[PAD_IDX] = 0.0
    wpad = np.zeros((vocab, HIDDEN), np.float32)
    wpad[: min(INPUT_DIM + 1, vocab)] = emb[: min(INPUT_DIM + 1, vocab)]
    if nchunk == 7:
        wpad[vocab - 1] = np.asarray(bias1, np.float32)
    wpad = wpad.reshape(nchunk, 128, HIDDEN).transpose(1, 0, 2)

    fc2t = np.asarray(fc2_w, np.float32).T  # [1024, 32]
    fc2t = fc2t.reshape(8, 128, H2).transpose(1, 0, 2)  # [128, 8, 32]

    off_w = 0
    off_fc2 = nchunk * HIDDEN
    wc_cols = off_fc2 + 8 * H2

    wconst = np.zeros((128, wc_cols), np.float16)
    wconst[:, off_w : off_w + nchunk * HIDDEN] = wpad.reshape(128, -1).astype(np.float16)
    wconst[:, off_fc2 : off_fc2 + 8 * H2] = fc2t.reshape(128, -1).astype(np.float16)

    stacked = np.zeros((32, H2), np.float32)
    stacked_b = np.zeros((32,), np.float32)
    stacked[0:8] = np.asarray(cp_w, np.float32)
    stacked_b[0:8] = np.asarray(cp_b, np.float32)
    for k in range(3):
        for u in range(8):
            stacked[8 + 8 * k + u] = np.asarray(wdl_w, np.float32)[3 * u + k]
            stacked_b[8 + 8 * k + u] = np.asarray(wdl_b, np.float32)[3 * u + k]

    smconst = np.zeros((32, SM_COLS), np.float16)
    smconst[:, SM_HW : SM_HW + 32] = stacked.T.astype(np.float16)  # [H2, 32]
    smconst[:, SM_FC2B] = np.asarray(fc2_b, np.float32).astype(np.float16)
    smconst[:, SM_HEADB] = stacked_b.astype(np.float16)
    uu = np.arange(32) % 8
    smconst[:, SM_LO] = np.where(uu == 0, -30000.0, uu).astype(np.float16)
    smconst[:, SM_HI] = np.where(uu == 7, 30000.0, uu + 1).astype(np.float16)
    sel = np.zeros((32, 4), np.float16)
    sel[np.arange(32), np.arange(32) // 8] = 1.0
    smconst[:, SM_SEL : SM_SEL + 4] = sel

    return dict(wconst=wconst, smconst=smconst)


def _prep_core(x_c, pc_c):
    BL = x_c.shape[0]
    T = BL // 128
    x16 = np.ascontiguousarray(
        np.asarray(x_c, np.int64)
        .astype(np.float16)
        .reshape(T, 128, NSLOT)
        .transpose(1, 0, 2)
    )
    pcrep = np.broadcast_to(
        np.asarray(pc_c, np.int64).astype(np.float16)[None, :], (32, BL)
    ).copy()
    return dict(x16=x16, pcrep=pcrep)


_NC_CACHE = {}


def kernel(x, piece_count, emb_w, bias1, fc2_w, fc2_b, cp_w, cp_b, wdl_w, wdl_b):
    x = np.asarray(x)
    piece_count = np.asarray(piece_count)
    B = x.shape[0]
    BL = B // NCORES
    nchunk = 6 if not np.any(np.asarray(bias1)) else 7

    key = (BL, nchunk)
    if key not in _NC_CACHE:
        _NC_CACHE[key] = build_nc(BL, nchunk)
    nc = _NC_CACHE[key]

    shared = _prep_shared(emb_w, bias1, fc2_w, fc2_b, cp_w, cp_b, wdl_w, wdl_b, nchunk)
    in_maps = []
    for c in range(NCORES):
        m = dict(shared)
        m.update(_prep_core(x[c * BL : (c + 1) * BL], piece_count[c * BL : (c + 1) * BL]))
        in_maps.append(m)

    res = bass_utils.run_bass_kernel_spmd(nc, in_maps, list(range(NCORES))).results
    out4 = np.concatenate([res[c]["out4"] for c in range(NCORES)], axis=1)  # [4, B]
    outT = out4.T.astype(np.float32)
    cp_out = np.ascontiguousarray(outT[:, 0:1])
    wdl_out = np.ascontiguousarray(outT[:, 1:4])
    return cp_out, wdl_out
